# revision 1
# baseline (speedup 1.0000x reference)
"""HSA (hierarchical splat attention) Bass kernel for Trainium2, 8 NeuronCores.

Math (per batch b):
    q = query @ Wq.T + bq                      [S, D]
    v = value @ Wv.T + bv                      [S, D]
    d2[s,n]  = |q_s|^2 - 2 q_s.c_n + |c_n|^2
    G[s,n]   = exp(-d2[s,n] * inv2v[n]),  inv2v = 0.5*exp(-2*log_scales)
    Asym[s,t]= sum_n G[s,n]*amp[n]*G[t,n]      (symmetric!)
    A        = Asym / (rowsum(Asym) + eps)
    out      = A @ v ;  y = out @ Wo.T + bo

Sharding: 8 cores = (batch b = c//2, seq-half h = c%2). Each core computes the
full-batch q-projection/G/v (needed for its rows of A) and its own 1024 output
rows. No collectives. Host pre-transposes inputs so every matmul has its
natural lhsT/rhs layout; the sequence axis is rolled per-core so "own" rows are
always columns 0..1023 (valid since A@v and rowsum are permutation-invariant
over t, and the q-side order is rolled consistently).

Device dataflow (all matmuls are lhsT.T @ rhs, K on partitions):
  qT[e,s]   : lhsT=Wq.T chunk,  rhs=xqT chunk           (accum over d)
  d2T[n,s]  : lhsT=(-2C).T,     rhs=qT   (+ ones64 lhsT, rhs=qT^2 -> |q|^2)
  GT,GampT  : ACT exp with per-partition scale=-inv2v, bias=-inv2v*c2 (+ln amp)
  v[t,e]    : lhsT=xvT chunk,   rhs=Wv.T chunk          (accum over d)
  AsymT[t,s]: lhsT=GT t-chunk,  rhs=GampT own-s   (K=64, one shot)
  rs[s]     : lhsT=ones128,     rhs=AsymT               (accum over t)
  outT[d,s] : lhsT=v d-slice,   rhs=AsymT               (accum over t)
  normalize : outT *= 1/(rs+eps)   (free-dim broadcast tiles)
  y[s,e]    : lhsT=outT s-slice, rhs=Wo.T chunk + bo    (accum over d)
"""

import numpy as np
import ml_dtypes

BF16 = ml_dtypes.bfloat16
EMBED = 1024
S = 2048
NSPL = 64
B = 4
NCORES = 8
P = 128
KC = EMBED // P   # 8 contraction chunks over d/e
TCH = S // P      # 16 t-chunks
SOWN = S // 2     # 1024 own output rows per core
SCH = SOWN // P   # 8
EPS = 1e-8

_PROG = None  # cached (nc, input_names)


def _build_program():
    import concourse.bass as bass
    import concourse.mybir as mybir
    from concourse import bacc
    from concourse.tile import TileContext
    from concourse.bass import ts, ds

    f32 = mybir.dt.float32
    bf16 = mybir.dt.bfloat16
    AF = mybir.ActivationFunctionType

    nc = bacc.Bacc("TRN2", target_bir_lowering=False, debug=False)
    xqT = nc.declare_dram_parameter("xqT", [EMBED, S], bf16, isOutput=False)
    xvT = nc.declare_dram_parameter("xvT", [EMBED, S], bf16, isOutput=False)
    wqT = nc.declare_dram_parameter("wqT", [EMBED, EMBED], bf16, isOutput=False)
    wvT = nc.declare_dram_parameter("wvT", [EMBED, EMBED], bf16, isOutput=False)
    woT = nc.declare_dram_parameter("woT", [EMBED, EMBED], bf16, isOutput=False)
    ctm2 = nc.declare_dram_parameter("ctm2", [EMBED, NSPL], bf16, isOutput=False)
    bq2 = nc.declare_dram_parameter("bq2", [P, KC], f32, isOutput=False)
    bvb = nc.declare_dram_parameter("bvb", [P, EMBED], f32, isOutput=False)
    bob = nc.declare_dram_parameter("bob", [P, EMBED], f32, isOutput=False)
    scn = nc.declare_dram_parameter("scn", [NSPL, 1], f32, isOutput=False)
    bgn = nc.declare_dram_parameter("bgn", [NSPL, 1], f32, isOutput=False)
    bgan = nc.declare_dram_parameter("bgan", [NSPL, 1], f32, isOutput=False)
    one64 = nc.declare_dram_parameter("one64", [P, NSPL], bf16, isOutput=False)
    one128 = nc.declare_dram_parameter("one128", [P, P], bf16, isOutput=False)
    y = nc.declare_dram_parameter("y", [SOWN, EMBED], f32, isOutput=True)

    with TileContext(nc) as tc:
        cpool_cm = tc.tile_pool(name="const", bufs=1)
        cpool = cpool_cm.__enter__()
        bq_sb = cpool.tile([P, KC], f32)
        bv_sb = cpool.tile([P, EMBED], f32)
        bo_sb = cpool.tile([P, EMBED], f32)
        sc_sb = cpool.tile([NSPL, 1], f32)
        bg_sb = cpool.tile([NSPL, 1], f32)
        bga_sb = cpool.tile([NSPL, 1], f32)
        o64_sb = cpool.tile([P, NSPL], bf16)
        o128_sb = cpool.tile([P, P], bf16)
        ct_sb = cpool.tile([P, KC, NSPL], bf16)
        gt = cpool.tile([NSPL, S], bf16)
        gamp = cpool.tile([NSPL, SOWN], bf16)

        nc.sync.dma_start(bq_sb[:], bq2[:])
        nc.sync.dma_start(sc_sb[:], scn[:])
        nc.sync.dma_start(bg_sb[:], bgn[:])
        nc.sync.dma_start(bga_sb[:], bgan[:])
        nc.sync.dma_start(o64_sb[:], one64[:])
        nc.sync.dma_start(o128_sb[:], one128[:])
        ctr = ctm2.rearrange("(k p) n -> k p n", p=P)
        for k in range(KC):
            nc.sync.dma_start(ct_sb[:, k], ctr[k])

        # ---------------- Phase A: q projection + G ----------------
        with tc.tile_pool(name="pa", bufs=1) as pa, \
             tc.tile_pool(name="qe", bufs=3) as qep, \
             tc.tile_pool(name="sqe", bufs=3) as sqp, \
             tc.tile_pool(name="psq", bufs=4, space="PSUM") as psq, \
             tc.tile_pool(name="psd2", bufs=4, space="PSUM") as psd2:
            xq = pa.tile([P, KC, S], bf16)
            wq = pa.tile([P, KC, EMBED], bf16)
            wqr = wqT.rearrange("(k p) e -> k p e", p=P)
            xqr = xqT.rearrange("(k p) s -> k p s", p=P)
            for k in range(KC):
                nc.sync.dma_start(wq[:, k], wqr[k])
                nc.sync.dma_start(xq[:, k], xqr[k])
            nc.sync.dma_start(bv_sb[:], bvb[:])
            nc.sync.dma_start(bo_sb[:], bob[:])
            d2ps = [psd2.tile([NSPL, 512], f32, tag="d2", name=f"d2ps{i}") for i in range(4)]
            for e in range(KC):
                qps = [psq.tile([P, 512], f32, tag="qps", name=f"qps{e}_{i}") for i in range(4)]
                for k in range(KC):
                    for s4 in range(4):
                        nc.tensor.matmul(
                            qps[s4], wq[:, k, ts(e, P)], xq[:, k, ts(s4, 512)],
                            start=(k == 0), stop=(k == KC - 1))
                qe = qep.tile([P, S], bf16, tag="qe")
                for s4 in range(4):
                    if s4 % 2 == 0:
                        nc.scalar.activation(qe[:, ts(s4, 512)], qps[s4],
                                             AF.Identity, bias=bq_sb[:, ds(e, 1)])
                    else:
                        nc.vector.tensor_scalar_add(qe[:, ts(s4, 512)], qps[s4],
                                                    bq_sb[:, ds(e, 1)])
                sq = sqp.tile([P, S], bf16, tag="sq")
                nc.vector.tensor_mul(sq, qe, qe)
                for s4 in range(4):
                    nc.tensor.matmul(d2ps[s4], ct_sb[:, e], qe[:, ts(s4, 512)],
                                     start=(e == 0), stop=False)
                for s4 in range(4):
                    nc.tensor.matmul(d2ps[s4], o64_sb[:], sq[:, ts(s4, 512)],
                                     start=False, stop=(e == KC - 1))
            for s4 in range(4):
                nc.scalar.activation(gt[:, ts(s4, 512)], d2ps[s4], AF.Exp,
                                     bias=bg_sb[:], scale=sc_sb[:])
            for s2 in range(2):
                nc.scalar.activation(gamp[:, ts(s2, 512)], d2ps[s2], AF.Exp,
                                     bias=bga_sb[:], scale=sc_sb[:])

        # ---------------- Phase B: v projection ----------------
        vpool_cm = tc.tile_pool(name="vpool", bufs=1)
        vpool = vpool_cm.__enter__()
        v_sb = vpool.tile([P, TCH, EMBED], bf16)
        with tc.tile_pool(name="pb", bufs=1) as pb, \
             tc.tile_pool(name="psv", bufs=3, space="PSUM") as psv:
            xv = pb.tile([P, KC, S], bf16)
            wv = pb.tile([P, KC, EMBED], bf16)
            wvr = wvT.rearrange("(k p) e -> k p e", p=P)
            xvr = xvT.rearrange("(k p) s -> k p s", p=P)
            for k in range(KC):
                nc.sync.dma_start(wv[:, k], wvr[k])
                nc.sync.dma_start(xv[:, k], xvr[k])
            for t in range(TCH):
                vps = psv.tile([P, EMBED], f32, tag="vps")
                for k in range(KC):
                    for eh in range(2):
                        nc.tensor.matmul(
                            vps[:, ts(eh, 512)], xv[:, k, ts(t, P)],
                            wv[:, k, ts(eh, 512)],
                            start=(k == 0), stop=(k == KC - 1))
                nc.vector.tensor_add(v_sb[:, t], vps, bv_sb)

        # ---------------- Phase C+D fused: Asym, rowsum, outT ----------------
        wpool_cm = tc.tile_pool(name="wpool", bufs=1)
        wpool = wpool_cm.__enter__()
        wo = wpool.tile([P, KC, EMBED], bf16)
        wor = woT.rearrange("(k p) e -> k p e", p=P)
        for k in range(KC):
            nc.sync.dma_start(wo[:, k], wor[k])
        otpool_cm = tc.tile_pool(name="otpool", bufs=1)
        otpool = otpool_cm.__enter__()
        outT = otpool.tile([P, KC, SOWN], bf16)

        with tc.tile_pool(name="asym", bufs=4) as asp, \
             tc.tile_pool(name="rssb", bufs=2) as rsp, \
             tc.tile_pool(name="psas", bufs=2, space="PSUM") as psas, \
             tc.tile_pool(name="pso", bufs=4, space="PSUM") as pso, \
             tc.tile_pool(name="psrs", bufs=1, space="PSUM") as psrs:
            for st in range(2):          # own-s tiles of 512
                rsps = psrs.tile([P, 512], f32, tag="rs")
                rsin = None
                for dh in range(2):      # d-chunk halves (4 each)
                    ops = [pso.tile([P, 512], f32, tag="ops", name=f"ops{st}_{dh}_{i}") for i in range(4)]
                    for t in range(TCH):
                        aps = psas.tile([P, 512], f32, tag="aps")
                        nc.tensor.matmul(aps, gt[:, ts(t, P)],
                                         gamp[:, ts(st, 512)],
                                         start=True, stop=True)
                        asy = asp.tile([P, 512], bf16, tag="asy")
                        if t % 2 == 0:
                            nc.vector.tensor_copy(asy, aps)
                        else:
                            nc.scalar.activation(asy, aps, AF.Copy)
                        if dh == 0:
                            nc.tensor.matmul(rsps, o128_sb[:], asy,
                                             start=(t == 0), stop=(t == TCH - 1))
                        for i in range(4):
                            d = dh * 4 + i
                            nc.tensor.matmul(ops[i], v_sb[:, t, ts(d, P)], asy,
                                             start=(t == 0), stop=(t == TCH - 1))
                    if dh == 0:
                        rs_sb = rsp.tile([P, 512], f32, tag="rss")
                        nc.vector.tensor_scalar_add(rs_sb, rsps, EPS)
                        rsin = rsp.tile([P, 512], f32, tag="rsin")
                        nc.vector.reciprocal(rsin, rs_sb)
                    for i in range(4):
                        d = dh * 4 + i
                        nc.vector.tensor_mul(outT[:, d, ds(st * 512, 512)],
                                             ops[i], rsin)

        # ---------------- Phase E: output projection ----------------
        with tc.tile_pool(name="ybuf", bufs=2) as yb, \
             tc.tile_pool(name="psy", bufs=3, space="PSUM") as psy:
            yr = y.rearrange("(c p) e -> c p e", p=P)
            for sc in range(SCH):
                yps = psy.tile([P, EMBED], f32, tag="yps")
                for k in range(KC):
                    for eh in range(2):
                        nc.tensor.matmul(
                            yps[:, ts(eh, 512)], outT[:, k, ts(sc, P)],
                            wo[:, k, ts(eh, 512)],
                            start=(k == 0), stop=(k == KC - 1))
                ysb = yb.tile([P, EMBED], f32, tag="ysb")
                nc.vector.tensor_add(ysb, yps, bo_sb)
                nc.sync.dma_start(yr[sc], ysb)
        otpool_cm.__exit__(None, None, None)
        wpool_cm.__exit__(None, None, None)
        vpool_cm.__exit__(None, None, None)
        cpool_cm.__exit__(None, None, None)

    nc.finalize()
    return nc


def _prep_inputs(query, key, value, Wq, bq, Wk, bk, Wv, bv, Wo, bo,
                 splat_centers, splat_log_scales, splat_amplitudes):
    """Build the 8 per-core input maps (host-side sharding/layout prep)."""
    f = np.float32
    q = np.asarray(query, f)
    v = np.asarray(value, f)
    Wq = np.asarray(Wq, f); bq = np.asarray(bq, f)
    Wv = np.asarray(Wv, f); bv = np.asarray(bv, f)
    Wo = np.asarray(Wo, f); bo = np.asarray(bo, f)
    C = np.asarray(splat_centers, f)
    ls = np.asarray(splat_log_scales, f)
    amp = np.asarray(splat_amplitudes, f)

    wqT = np.ascontiguousarray(Wq.T).astype(BF16)
    wvT = np.ascontiguousarray(Wv.T).astype(BF16)
    woT = np.ascontiguousarray(Wo.T).astype(BF16)
    ctm2 = np.ascontiguousarray((-2.0 * C).T).astype(BF16)
    bq2 = np.ascontiguousarray(bq.reshape(KC, P).T)
    bvb = np.ascontiguousarray(np.broadcast_to(bv, (P, EMBED)))
    bob = np.ascontiguousarray(np.broadcast_to(bo, (P, EMBED)))
    inv2v = 0.5 * np.exp(-2.0 * ls).astype(f)
    c2 = (C.astype(np.float64) ** 2).sum(1)
    scn = (-inv2v).reshape(NSPL, 1).astype(f)
    bgn = (-inv2v * c2).reshape(NSPL, 1).astype(f)
    # fold amplitude into one G factor: amp*exp(x) = exp(x + ln amp)
    bgan = (-inv2v * c2 + np.log(np.maximum(amp, 1e-38))).reshape(NSPL, 1).astype(f)
    one64 = np.ones((P, NSPL), BF16)
    one128 = np.ones((P, P), BF16)

    shared = dict(wqT=wqT, wvT=wvT, woT=woT, ctm2=ctm2, bq2=bq2, bvb=bvb,
                  bob=bob, scn=scn, bgn=bgn, bgan=bgan, one64=one64,
                  one128=one128)
    in_maps = []
    for c in range(NCORES):
        b, h = c // 2, c % 2
        # roll the sequence axis so own rows are always 0..1023
        qb = np.concatenate([q[b, h * SOWN:], q[b, :h * SOWN]], axis=0)
        vb = np.concatenate([v[b, h * SOWN:], v[b, :h * SOWN]], axis=0)
        m = dict(shared)
        m["xqT"] = np.ascontiguousarray(qb.T).astype(BF16)
        m["xvT"] = np.ascontiguousarray(vb.T).astype(BF16)
        in_maps.append(m)
    return in_maps


def run_cores(inputs, trace=False):
    """Run the SPMD kernel; returns (full_output, BassKernelResults)."""
    global _PROG
    from concourse.bass_utils import run_bass_kernel_spmd
    if _PROG is None:
        _PROG = _build_program()
    nc = _PROG
    in_maps = _prep_inputs(**inputs)
    res = run_bass_kernel_spmd(nc, in_maps, list(range(NCORES)), trace=trace)
    out = np.empty((B, S, EMBED), np.float32)
    for c in range(NCORES):
        b, h = c // 2, c % 2
        out[b, h * SOWN:(h + 1) * SOWN] = res.results[c]["y"]
    return out, res


def kernel(**inputs):
    out, _ = run_cores(inputs, trace=False)
    return out



# revision 2
# speedup vs baseline: 2.3803x; 2.3803x over previous
"""HSA (hierarchical splat attention) Bass kernel for Trainium2, 8 NeuronCores.

Math (per batch b):
    q = query @ Wq.T + bq                      [S, D]
    d2[s,n]  = |q_s|^2 - 2 q_s.c_n + |c_n|^2
    G[s,n]   = exp(-d2[s,n] * inv2v[n]),  inv2v = 0.5*exp(-2*log_scales)
    A        = (G diag(amp) G^T) row-normalized (+eps)
    out      = A @ (value @ Wv.T + bv) ;  y = out @ Wo.T + bo

Because A = G diag(a) G^T is rank-64, A is never materialized:
    gsum[n]  = sum_t G[t,n]
    Hraw     = G^T @ value                       [N, D]
    M        = Hraw @ Wv.T @ Wo.T + gsum (x) w1  [N, D],  w1 = Wo@bv + bo
    rs[s]    = (amp*G)[s,:] @ gsum  (+ eps)
    y[s,:]   = ((amp*G)[s,:] @ M + eps*bo) / rs[s]
The eps*bo term makes the G-underflow case exact: rs=eps, y=bo.
bo and eps ride inside the matmuls via an appended ones-row in Ga
(row 64) matching an eps*bo row in M.

Sharding: 8 cores = (batch b = c//2, seq-half h = c%2), no collectives.
Each core computes full-batch q-proj/G (needed for gsum/Hraw) and its own
1024 output rows. The sequence axis is rolled per-core so own rows are
always t-chunks 0..7 (valid: the t-contractions are permutation-invariant).

Device dataflow (matmul = lhsT.T @ rhs, contraction on partitions):
  qT[e,s]    : lhsT=wq chunk, rhs=xq chunk          (accum over d)
  d2t[t,n]   : psum [128,8,64] x2; bank-wide K=1 init matmul injects
               -inv2v*c2 (start=True), then per e: lhsT=qe[:,tb] rhs=cts
               (+2*inv2v*q.c) and lhsT=sq[:,tb] rhs=o64s (-inv2v*|q|^2)
  G,Ga       : ACT exp (one [128,512] op per tile); Ga = G * ampw (DVE)
  gsum       : lhsT=ones col, rhs=G t-chunks -> [1,64]; PE-transpose -> [64,1]
  HrawT[e,n] : lhsT=vrl[t, e-chunk], rhs=G[t] t-chunk   (accum over t)
  HT[e',n]   : lhsT=wvT chunk,  rhs=HrawT chunk         (accum over d)
  M[n,e']    : lhsT=HT chunk,   rhs=woT chunk           (accum over e')
               + rank-1 gsum (x) w1 via DVE; -> m_sb [65,1024] (row64=eps*bo)
  rs         : psum [128,8]; K=1 init=eps; lhsT=GaT[:,sc], rhs=gsum col
  U,y        : lhsT=GaT[:,sc] [65,128], rhs=m_sb -> U; y = U * recip(rs)
"""

import numpy as np
import ml_dtypes

BF16 = ml_dtypes.bfloat16
EMBED = 1024
S = 2048
NSPL = 64
B = 4
NCORES = 8
P = 128
KC = EMBED // P   # 8 contraction chunks over d/e
TCH = S // P      # 16 t-chunks
SOWN = S // 2     # 1024 own output rows per core
SCH = SOWN // P   # 8
EPS = 1e-8

_PROG = None  # cached program


def _build_program():
    import concourse.bass as bass
    import concourse.mybir as mybir
    from concourse import bacc
    from concourse.tile import TileContext
    from concourse.bass import ts, ds

    f32 = mybir.dt.float32
    bf16 = mybir.dt.bfloat16
    AF = mybir.ActivationFunctionType

    nc = bacc.Bacc("TRN2", target_bir_lowering=False, debug=False)
    xqT = nc.declare_dram_parameter("xqT", [EMBED, S], bf16, isOutput=False)
    vrl = nc.declare_dram_parameter("vrl", [S, EMBED], bf16, isOutput=False)
    wqT = nc.declare_dram_parameter("wqT", [EMBED, EMBED], bf16, isOutput=False)
    wvT = nc.declare_dram_parameter("wvT", [EMBED, EMBED], bf16, isOutput=False)
    woT = nc.declare_dram_parameter("woT", [EMBED, EMBED], bf16, isOutput=False)
    cts = nc.declare_dram_parameter("cts", [EMBED, NSPL], bf16, isOutput=False)
    bq2 = nc.declare_dram_parameter("bq2", [P, KC], f32, isOutput=False)
    o64s = nc.declare_dram_parameter("o64s", [P, NSPL], bf16, isOutput=False)
    cb1w = nc.declare_dram_parameter("cb1w", [1, 512], f32, isOutput=False)
    ampw = nc.declare_dram_parameter("ampw", [P, 512], bf16, isOutput=False)
    w1b = nc.declare_dram_parameter("w1b", [NSPL, EMBED], f32, isOutput=False)
    epsbo = nc.declare_dram_parameter("epsbo", [1, EMBED], bf16, isOutput=False)
    epsrow = nc.declare_dram_parameter("epsrow", [1, SCH], bf16, isOutput=False)
    ones1f = nc.declare_dram_parameter("ones1f", [1, P], f32, isOutput=False)
    ones1b = nc.declare_dram_parameter("ones1b", [1, P], bf16, isOutput=False)
    onecol = nc.declare_dram_parameter("onecol", [P, 1], bf16, isOutput=False)
    one11f = nc.declare_dram_parameter("one11f", [1, 1], f32, isOutput=False)
    id128 = nc.declare_dram_parameter("id128", [P, P], bf16, isOutput=False)
    y = nc.declare_dram_parameter("y", [SOWN, EMBED], f32, isOutput=True)

    with TileContext(nc) as tc:
        cpool_cm = tc.tile_pool(name="const", bufs=1)
        cpool = cpool_cm.__enter__()
        bq_sb = cpool.tile([P, KC], f32)
        cts_sb = cpool.tile([P, KC, NSPL], bf16)
        o64s_sb = cpool.tile([P, NSPL], bf16)
        cb1w_sb = cpool.tile([1, 512], f32)
        ampw_sb = cpool.tile([P, 512], bf16)
        w1b_sb = cpool.tile([NSPL, EMBED], f32)
        epsr_sb = cpool.tile([1, SCH], bf16)
        on1f_sb = cpool.tile([1, P], f32)
        on1b_sb = cpool.tile([1, P], bf16)
        oncl_sb = cpool.tile([P, 1], bf16)
        o11f_sb = cpool.tile([1, 1], f32)
        id_sb = cpool.tile([P, P], bf16)
        gts = cpool.tile([P, TCH, NSPL], bf16)     # G in [t, n] layout
        ga_sb = cpool.tile([P, SCH, NSPL], bf16)   # amp*G, own rows
        gaT = cpool.tile([NSPL + 1, SCH, P], bf16)  # (amp*G)^T + ones row
        vrl_sb = cpool.tile([P, TCH, EMBED], bf16)
        wv_sb = cpool.tile([P, KC, EMBED], bf16)
        wo_sb = cpool.tile([P, KC, EMBED], bf16)
        hrawT_sb = cpool.tile([P, KC, NSPL], bf16)
        ht_sb = cpool.tile([P, KC, NSPL], bf16)
        m_sb = cpool.tile([NSPL + 1, EMBED], bf16)
        t1_sb = cpool.tile([NSPL, EMBED], f32)
        gsum_sb = cpool.tile([1, NSPL], f32)
        gsumc_sb = cpool.tile([NSPL, 1], f32)
        gse_sb = cpool.tile([NSPL + 1, 1], bf16)
        rs_sb = cpool.tile([P, SCH], f32)
        rcp_sb = cpool.tile([P, SCH], f32)

        nc.sync.dma_start(bq_sb[:], bq2[:])
        nc.sync.dma_start(o64s_sb[:], o64s[:])
        nc.sync.dma_start(cb1w_sb[:], cb1w[:])
        nc.sync.dma_start(ampw_sb[:], ampw[:])
        nc.sync.dma_start(epsr_sb[:], epsrow[:])
        nc.sync.dma_start(on1f_sb[:], ones1f[:])
        nc.sync.dma_start(on1b_sb[:], ones1b[:])
        nc.sync.dma_start(oncl_sb[:], onecol[:])
        nc.sync.dma_start(o11f_sb[:], one11f[:])
        nc.sync.dma_start(id_sb[:], id128[:])
        nc.sync.dma_start(w1b_sb[:], w1b[:])
        nc.sync.dma_start(m_sb[NSPL:NSPL + 1, :], epsbo[:])
        ctr = cts.rearrange("(k p) n -> k p n", p=P)
        for k in range(KC):
            nc.sync.dma_start(cts_sb[:, k], ctr[k])

        # ---------------- Phase A: q projection + d2 in [t, n] ----------------
        with tc.tile_pool(name="pa", bufs=1) as pa, \
             tc.tile_pool(name="qe", bufs=2) as qep, \
             tc.tile_pool(name="sqe", bufs=2) as sqp, \
             tc.tile_pool(name="psq", bufs=4, space="PSUM") as psq, \
             tc.tile_pool(name="psd", bufs=1, space="PSUM") as psd:
            xq = pa.tile([P, KC, S], bf16)
            wq = pa.tile([P, KC, EMBED], bf16)
            wqr = wqT.rearrange("(k p) e -> k p e", p=P)
            xqr = xqT.rearrange("(k p) s -> k p s", p=P)
            for k in range(KC):
                nc.sync.dma_start(wq[:, k], wqr[k])
                nc.sync.dma_start(xq[:, k], xqr[k])
            # prefetch Phase B data while A computes
            vrr = vrl.rearrange("(t p) e -> t p e", p=P)
            for t in range(TCH):
                nc.sync.dma_start(vrl_sb[:, t], vrr[t])
            wvr = wvT.rearrange("(k p) e -> k p e", p=P)
            wor = woT.rearrange("(k p) e -> k p e", p=P)
            for k in range(KC):
                nc.sync.dma_start(wv_sb[:, k], wvr[k])
                nc.sync.dma_start(wo_sb[:, k], wor[k])

            d2a = [psd.tile([P, SCH, NSPL], f32, name=f"d2a{i}")
                   for i in range(2)]
            # bank-wide group init: fills each d2 bank with -inv2v*c2 (K=1)
            for i in range(2):
                nc.tensor.matmul(d2a[i][:, :, :], on1f_sb[:], cb1w_sb[:],
                                 start=True, stop=False)

            prev = None  # software pipelining: d2(e-1) emitted after qps(e)
            for e in range(KC):
                qps = [psq.tile([P, 512], f32, tag="qps", name=f"qps{e}_{i}")
                       for i in range(4)]
                for k in range(KC):
                    for s4 in range(4):
                        nc.tensor.matmul(
                            qps[s4], wq[:, k, ts(e, P)], xq[:, k, ts(s4, 512)],
                            start=(k == 0), stop=(k == KC - 1))
                if prev is not None:
                    qep_prev, sqp_prev, eprev = prev
                    for tb in range(TCH):
                        sl = d2a[tb // SCH][:, tb % SCH]
                        nc.tensor.matmul(sl, qep_prev[:, ts(tb, P)],
                                         cts_sb[:, eprev], start=False,
                                         stop=False)
                        nc.tensor.matmul(sl, sqp_prev[:, ts(tb, P)],
                                         o64s_sb[:], start=False, stop=False)
                qe = qep.tile([P, S], bf16, tag="qe")
                for s4 in range(4):
                    if s4 % 2 == 0:
                        nc.scalar.activation(qe[:, ts(s4, 512)], qps[s4],
                                             AF.Identity, bias=bq_sb[:, ds(e, 1)])
                    else:
                        nc.vector.tensor_scalar_add(qe[:, ts(s4, 512)], qps[s4],
                                                    bq_sb[:, ds(e, 1)])
                sq = sqp.tile([P, S], bf16, tag="sq")
                nc.vector.tensor_mul(sq, qe, qe)
                prev = (qe, sq, e)
            qe, sq, e = prev
            for tb in range(TCH):
                sl = d2a[tb // SCH][:, tb % SCH]
                nc.tensor.matmul(sl, qe[:, ts(tb, P)], cts_sb[:, e],
                                 start=False, stop=False)
                nc.tensor.matmul(sl, sq[:, ts(tb, P)], o64s_sb[:],
                                 start=False, stop=(tb % SCH == SCH - 1))
            # G = exp(arg): one wide op per tile
            nc.scalar.activation(gts[:, 0:SCH], d2a[0][:], AF.Exp)
            nc.scalar.activation(gts[:, SCH:TCH], d2a[1][:], AF.Exp)

        # ---------------- Phase B: gsum, Ga^T, H chain ----------------
        nc.vector.tensor_mul(ga_sb[:], gts[:, 0:SCH], ampw_sb[:])
        with tc.tile_pool(name="gat", bufs=2, space="PSUM") as gat, \
             tc.tile_pool(name="gsp", bufs=1, space="PSUM") as gsp, \
             tc.tile_pool(name="hrt", bufs=2, space="PSUM") as hrt:
            gsps = gsp.tile([1, NSPL], f32, name="gsps")
            for tb in range(TCH):
                nc.tensor.matmul(gsps, oncl_sb[:], gts[:, tb],
                                 start=(tb == 0), stop=(tb == TCH - 1))
            nc.scalar.activation(gsum_sb[:], gsps, AF.Copy)
            gscps = gsp.tile([NSPL, 1], f32, name="gscps")
            nc.tensor.matmul(gscps, gsum_sb[:], o11f_sb[:], start=True,
                             stop=True)
            nc.vector.tensor_copy(gsumc_sb[:], gscps)
            nc.gpsimd.memset(gse_sb[:], 0.0)
            nc.vector.tensor_copy(gse_sb[0:NSPL], gscps)
            nc.gpsimd.memset(gaT[NSPL:NSPL + 1], 1.0)
            for sc in range(SCH):
                gatp = gat.tile([NSPL, P], bf16, tag="gat")
                nc.tensor.transpose(gatp, ga_sb[:, sc], id_sb[:])
                if sc % 2 == 0:
                    nc.vector.tensor_copy(gaT[0:NSPL, sc], gatp)
                else:
                    nc.scalar.activation(gaT[0:NSPL, sc], gatp, AF.Copy)
            # HrawT[e,n] = sum_t value[t,e] G[t,n]
            for e in range(KC):
                h = hrt.tile([P, NSPL], f32, tag="hrt")
                for t in range(TCH):
                    nc.tensor.matmul(h, vrl_sb[:, t, ts(e, P)], gts[:, t],
                                     start=(t == 0), stop=(t == TCH - 1))
                if e % 2 == 0:
                    nc.scalar.activation(hrawT_sb[:, e], h, AF.Copy)
                else:
                    nc.vector.tensor_copy(hrawT_sb[:, e], h)

        with tc.tile_pool(name="htp", bufs=2, space="PSUM") as htp, \
             tc.tile_pool(name="mp", bufs=1, space="PSUM") as mp:
            mps = [mp.tile([NSPL, 512], f32, name=f"mps{j}") for j in range(2)]
            for ec in range(KC):
                h2 = htp.tile([P, NSPL], f32, tag="htp")
                for d in range(KC):
                    nc.tensor.matmul(h2, wv_sb[:, d, ts(ec, P)],
                                     hrawT_sb[:, d], start=(d == 0),
                                     stop=(d == KC - 1))
                if ec % 2 == 0:
                    nc.vector.tensor_copy(ht_sb[:, ec], h2)
                else:
                    nc.scalar.activation(ht_sb[:, ec], h2, AF.Copy)
                for j in range(2):
                    nc.tensor.matmul(mps[j], ht_sb[:, ec],
                                     wo_sb[:, ec, ts(j, 512)],
                                     start=(ec == 0), stop=(ec == KC - 1))
            nc.vector.tensor_scalar_mul(t1_sb[:], w1b_sb[:], gsumc_sb[:])
            for j in range(2):
                nc.vector.tensor_add(m_sb[0:NSPL, ts(j, 512)], mps[j],
                                     t1_sb[:, ts(j, 512)])

        # ---------------- Phase C: rs, U, y ----------------
        with tc.tile_pool(name="rsp", bufs=1, space="PSUM") as rsp, \
             tc.tile_pool(name="ups", bufs=4, space="PSUM") as ups, \
             tc.tile_pool(name="yb", bufs=2) as yb:
            rsps = rsp.tile([P, SCH], f32, name="rsps")
            nc.tensor.matmul(rsps, on1b_sb[:], epsr_sb[:], start=True,
                             stop=False)
            for sc in range(SCH):
                nc.tensor.matmul(rsps[:, ds(sc, 1)], gaT[:, sc], gse_sb[:],
                                 start=False, stop=(sc == SCH - 1))
            nc.vector.tensor_copy(rs_sb[:], rsps)
            nc.vector.reciprocal(rcp_sb[:], rs_sb[:])
            yr = y.rearrange("(c p) e -> c p e", p=P)
            for sc in range(SCH):
                ua = ups.tile([P, 512], f32, tag="ups", name=f"ua{sc}")
                ub = ups.tile([P, 512], f32, tag="ups", name=f"ub{sc}")
                nc.tensor.matmul(ua, gaT[:, sc], m_sb[:, ts(0, 512)],
                                 start=True, stop=True)
                nc.tensor.matmul(ub, gaT[:, sc], m_sb[:, ts(1, 512)],
                                 start=True, stop=True)
                ysb = yb.tile([P, EMBED], f32, tag="ysb")
                nc.scalar.activation(ysb[:, ts(0, 512)], ua, AF.Copy,
                                     scale=rcp_sb[:, ds(sc, 1)])
                nc.vector.tensor_scalar_mul(ysb[:, ts(1, 512)], ub,
                                            rcp_sb[:, ds(sc, 1)])
                nc.sync.dma_start(yr[sc], ysb)
        cpool_cm.__exit__(None, None, None)

    nc.finalize()
    return nc


def _prep_inputs(query, key, value, Wq, bq, Wk, bk, Wv, bv, Wo, bo,
                 splat_centers, splat_log_scales, splat_amplitudes):
    """Build the 8 per-core input maps (host-side sharding/layout prep)."""
    f = np.float32
    q = np.asarray(query, f)
    v = np.asarray(value, f)
    Wq = np.asarray(Wq, f); bq = np.asarray(bq, f)
    Wv = np.asarray(Wv, f); bv = np.asarray(bv, f)
    Wo = np.asarray(Wo, f); bo = np.asarray(bo, f)
    C = np.asarray(splat_centers, f)
    ls = np.asarray(splat_log_scales, f)
    amp = np.asarray(splat_amplitudes, f)

    wqT = np.ascontiguousarray(Wq.T).astype(BF16)
    wvT = np.ascontiguousarray(Wv.T).astype(BF16)
    woT = np.ascontiguousarray(Wo.T).astype(BF16)
    bq2 = np.ascontiguousarray(bq.reshape(KC, P).T)
    inv2v = (0.5 * np.exp(-2.0 * ls)).astype(np.float64)
    c2 = (C.astype(np.float64) ** 2).sum(1)
    # exponent arg = -inv2v*d2 = (2*inv2v)*q.c + (-inv2v)*|q|^2 + (-inv2v*c2)
    cts = np.ascontiguousarray((2.0 * inv2v[:, None] * C).T).astype(BF16)
    o64s = np.broadcast_to((-inv2v).astype(f), (P, NSPL)).astype(BF16)
    o64s = np.ascontiguousarray(o64s)
    cb1w = np.tile((-inv2v * c2).astype(f), SCH).reshape(1, 512)
    cb1w = np.ascontiguousarray(cb1w.astype(f))
    ampw = np.tile(amp.astype(f), SCH).reshape(1, 512)
    ampw = np.ascontiguousarray(np.broadcast_to(ampw, (P, 512))).astype(BF16)
    w1 = (Wo.astype(np.float64) @ bv.astype(np.float64) + bo).astype(f)
    w1b = np.ascontiguousarray(np.broadcast_to(w1, (NSPL, EMBED)))
    epsbo = np.ascontiguousarray((EPS * bo).reshape(1, EMBED)).astype(BF16)
    epsrow = np.full((1, SCH), EPS, BF16)
    ones1f = np.ones((1, P), f)
    ones1b = np.ones((1, P), BF16)
    onecol = np.ones((P, 1), BF16)
    one11f = np.ones((1, 1), f)
    id128 = np.eye(P, dtype=BF16)

    shared = dict(wqT=wqT, wvT=wvT, woT=woT, bq2=bq2, cts=cts, o64s=o64s,
                  cb1w=cb1w, ampw=ampw, w1b=w1b, epsbo=epsbo, epsrow=epsrow,
                  ones1f=ones1f, ones1b=ones1b, onecol=onecol, one11f=one11f,
                  id128=id128)
    in_maps = []
    for c in range(NCORES):
        b, h = c // 2, c % 2
        # roll the sequence axis so own rows are always 0..1023
        qb = np.concatenate([q[b, h * SOWN:], q[b, :h * SOWN]], axis=0)
        vb = np.concatenate([v[b, h * SOWN:], v[b, :h * SOWN]], axis=0)
        m = dict(shared)
        m["xqT"] = np.ascontiguousarray(qb.T).astype(BF16)
        m["vrl"] = np.ascontiguousarray(vb).astype(BF16)
        in_maps.append(m)
    return in_maps


def run_cores(inputs, trace=False):
    """Run the SPMD kernel; returns (full_output, BassKernelResults)."""
    global _PROG
    from concourse.bass_utils import run_bass_kernel_spmd
    if _PROG is None:
        _PROG = _build_program()
    nc = _PROG
    in_maps = _prep_inputs(**inputs)
    res = run_bass_kernel_spmd(nc, in_maps, list(range(NCORES)), trace=trace)
    out = np.empty((B, S, EMBED), np.float32)
    for c in range(NCORES):
        b, h = c // 2, c % 2
        out[b, h * SOWN:(h + 1) * SOWN] = res.results[c]["y"]
    return out, res


def kernel(**inputs):
    out, _ = run_cores(inputs, trace=False)
    return out


# revision 32
# speedup vs baseline: 3.2178x; 1.3518x over previous
"""HSA (hierarchical splat attention) Bass kernel for Trainium2, 8 NeuronCores.

Math (per batch b):
    q = query @ Wq.T + bq                      [S, D]
    d2[s,n]  = |q_s|^2 - 2 q_s.c_n + |c_n|^2
    G[s,n]   = exp(-d2[s,n] * inv2v[n]),  inv2v = 0.5*exp(-2*log_scales)
    A        = (G diag(amp) G^T) row-normalized (+eps)
    out      = A @ (value @ Wv.T + bv) ;  y = out @ Wo.T + bo

Because A = G diag(a) G^T is rank-64, A is never materialized:
    gsum[n]  = sum_t G[t,n]
    Hraw     = G^T @ value                       [N, D]
    M        = Hraw @ Wv.T @ Wo.T + gsum (x) w1  [N, D],  w1 = Wo@bv + bo
    rs[s]    = (amp*G)[s,:] @ gsum  (+ eps)
    y[s,:]   = ((amp*G)[s,:] @ M + eps*bo) / rs[s]
The eps*bo term makes the G-underflow case exact: rs=eps, y=bo.
bo and eps ride inside the matmuls via an appended ones-row in Ga
(row 64) matching an eps*bo row in M.

Sharding: 8 cores = (batch b = c//2, seq-half h = c%2), no collectives.
Each core computes full-batch q-proj/G (needed for gsum/Hraw) and its own
1024 output rows. The sequence axis is rolled per-core so own rows are
always t-chunks 0..7 (valid: the t-contractions are permutation-invariant).

Device dataflow (matmul = lhsT.T @ rhs, contraction on partitions):
  qT[e,s]    : lhsT=wq chunk, rhs=xq chunk          (accum over d)
  d2t[t,n]   : psum [128,8,64] x2; bank-wide K=1 init matmul injects
               -inv2v*c2 (start=True), then per e: lhsT=qe[:,tb] rhs=cts
               (+2*inv2v*q.c) and lhsT=sq[:,tb] rhs=o64s (-inv2v*|q|^2)
  G,Ga       : ACT exp (one [128,512] op per tile); Ga = G * ampw (DVE)
  gsum       : lhsT=ones col, rhs=G t-chunks -> [1,64]; PE-transpose -> [64,1]
  HrawT[e,n] : lhsT=vrl[t, e-chunk], rhs=G[t] t-chunk   (accum over t)
  HT[e',n]   : lhsT=wvT chunk,  rhs=HrawT chunk         (accum over d)
  M[n,e']    : lhsT=HT chunk,   rhs=woT chunk           (accum over e')
               + rank-1 gsum (x) w1 via DVE; -> m_sb [65,1024] (row64=eps*bo)
  rs         : psum [128,8]; K=1 init=eps; lhsT=GaT[:,sc], rhs=gsum col
  U,y        : lhsT=GaT[:,sc] [65,128], rhs=m_sb -> U; y = U * recip(rs)
"""

import numpy as np
import ml_dtypes

BF16 = ml_dtypes.bfloat16
EMBED = 1024
S = 2048
NSPL = 64
B = 4
NCORES = 8
P = 128
KC = EMBED // P   # 8 contraction chunks over d/e
TCH = S // P      # 16 t-chunks
SOWN = S // 2     # 1024 own output rows per core
SCH = SOWN // P   # 8
EPS = 1e-8

_PROG = None  # cached program


def _build_program():
    import concourse.bass as bass
    import concourse.mybir as mybir
    from concourse import bacc
    from concourse.tile import TileContext
    from concourse.bass import ts, ds

    f32 = mybir.dt.float32
    bf16 = mybir.dt.bfloat16
    AF = mybir.ActivationFunctionType

    nc = bacc.Bacc("TRN2", target_bir_lowering=False, debug=False)
    xqT = nc.declare_dram_parameter("xqT", [EMBED, S], bf16, isOutput=False)
    vrl = nc.declare_dram_parameter("vrl", [S, EMBED], bf16, isOutput=False)
    wqT = nc.declare_dram_parameter("wqT", [EMBED, EMBED], bf16, isOutput=False)
    wvT = nc.declare_dram_parameter("wvT", [EMBED, EMBED], bf16, isOutput=False)
    woT = nc.declare_dram_parameter("woT", [EMBED, EMBED], bf16, isOutput=False)
    cts = nc.declare_dram_parameter("cts", [EMBED, NSPL], bf16, isOutput=False)
    bq2 = nc.declare_dram_parameter("bq2", [P, KC], f32, isOutput=False)
    # packed constants: fewer DMA instructions (HWDGE serializes per-DMA)
    # blob_b [128, 193] bf16: o64s(64) | id128(128) | onecol(1)
    blob_b = nc.declare_dram_parameter("blob_b", [P, 193], bf16, isOutput=False)
    # blob1b [1, 648] bf16: ones(128) | epsrow(8) | zeros(512)
    blob1b = nc.declare_dram_parameter("blob1b", [1, 648], bf16, isOutput=False)
    # blob1f [1, 641] f32: cb1w(512) | ones(128) | one(1)
    blob1f = nc.declare_dram_parameter("blob1f", [1, 641], f32, isOutput=False)
    # w1b [64, 1025] f32: broadcast (Wo@bv + bo) | amp column
    w1b = nc.declare_dram_parameter("w1b", [NSPL, EMBED + 1], f32,
                                    isOutput=False)
    # eps*bo split hi/lo so the bf16 rank-1 rows carry ~16 mantissa bits
    epsbo = nc.declare_dram_parameter("epsbo", [2, EMBED], bf16, isOutput=False)
    y = nc.declare_dram_parameter("y", [SOWN, EMBED], bf16, isOutput=True)

    with TileContext(nc) as tc:
        cpool_cm = tc.tile_pool(name="const", bufs=1)
        cpool = cpool_cm.__enter__()
        bq_sb = cpool.tile([P, KC], f32)
        cts_sb = cpool.tile([P, KC, NSPL], bf16)
        bb_sb = cpool.tile([P, 193], bf16)
        b1b_sb = cpool.tile([1, 648], bf16)
        b1f_sb = cpool.tile([1, 641], f32)
        w1b_sb = cpool.tile([NSPL, EMBED + 1], f32)
        gts = cpool.tile([P, TCH, NSPL], bf16)     # G in [t, n] layout
        gaT = cpool.tile([NSPL + 2, SCH, P], bf16)  # G^T own rows + ones rows
        vrl_sb = cpool.tile([P, TCH, EMBED], bf16)
        wv_sb = cpool.tile([P, KC, EMBED], bf16)
        wo_sb = cpool.tile([P, KC, EMBED], bf16)
        hrawT_sb = cpool.tile([P, KC, NSPL], bf16)
        ht_sb = cpool.tile([P, KC, NSPL], bf16)
        m_sb = cpool.tile([NSPL + 2, EMBED], bf16)
        t1_sb = cpool.tile([NSPL, EMBED], f32)
        gsum_sb = cpool.tile([1, NSPL], f32)
        gsumc_sb = cpool.tile([NSPL, 1], f32)
        gsa_sb = cpool.tile([NSPL, 1], f32)        # amp * gsum
        gse_sb = cpool.tile([NSPL + 2, 1], bf16)
        rs_sb = cpool.tile([P, SCH], f32)
        rcp_sb = cpool.tile([P, SCH], f32)

        # const views into packed blobs
        o64s_sb = bb_sb[:, 0:NSPL]
        id_sb = bb_sb[:, NSPL:NSPL + P]
        oncl_sb = bb_sb[:, 192:193]
        on1b_sb = b1b_sb[:, 0:P]
        epsr_sb = b1b_sb[:, P:P + SCH]
        zrow_sb = b1b_sb[:, 136:648]
        cb1w_sb = b1f_sb[:, 0:512]
        on1f_sb = b1f_sb[:, 512:512 + P]
        o11f_sb = b1f_sb[:, 640:641]
        ampc_sb = w1b_sb[:, EMBED:EMBED + 1]

        # ---------------- Phase A: q projection + d2 in [t, n] ----------------
        with tc.tile_pool(name="pa", bufs=1) as pa, \
             tc.tile_pool(name="qe", bufs=2) as qep, \
             tc.tile_pool(name="sqe", bufs=2) as sqp, \
             tc.tile_pool(name="psq", bufs=4, space="PSUM") as psq, \
             tc.tile_pool(name="psd", bufs=1, space="PSUM") as psd:
            xq = pa.tile([P, KC, S], bf16)
            wq = pa.tile([P, KC, EMBED], bf16)
            wqr = wqT.rearrange("(k p) e -> k p e", p=P)
            xqr = xqT.rearrange("(k p) s -> k p s", p=P)
            # critical-path chunks spread over the SP/Activation/Pool queues
            # (v1 charges transfer time to the issuing engine); k=0 split
            # into small pieces so the first matmuls start ASAP
            nc.sync.dma_start(wq[:, 0, 0:512], wqr[0][:, 0:512])
            nc.sync.dma_start(xq[:, 0, 0:512], xqr[0][:, 0:512])
            nc.sync.dma_start(wq[:, 0, 512:EMBED], wqr[0][:, 512:EMBED])
            nc.sync.dma_start(xq[:, 0, 512:S], xqr[0][:, 512:S])
            # b1f first: the d2 psum-init matmuls read cb1w from it
            nc.gpsimd.dma_start(b1f_sb[:], blob1f[:])
            qeng = {1: nc.scalar, 2: nc.gpsimd, 3: nc.sync, 4: nc.scalar,
                    5: nc.gpsimd, 6: nc.sync, 7: nc.scalar}
            for k in range(1, KC):
                qeng[k].dma_start(wq[:, k], wqr[k])
                qeng[k].dma_start(xq[:, k], xqr[k])
            # remaining constants + bulk prefetch on the Pool engine
            nc.gpsimd.dma_start(bq_sb[:], bq2[:])
            nc.gpsimd.dma_start(cts_sb[:], cts.rearrange("(k p) n -> p k n", p=P))
            nc.gpsimd.dma_start(bb_sb[:], blob_b[:])
            nc.gpsimd.dma_start(b1b_sb[:], blob1b[:])
            nc.gpsimd.dma_start(m_sb[NSPL:NSPL + 2, :], epsbo[:])
            nc.gpsimd.dma_start(w1b_sb[:], w1b[:])
            nc.gpsimd.dma_start(vrl_sb[:], vrl.rearrange("(t p) e -> p t e", p=P))
            nc.gpsimd.dma_start(wv_sb[:], wvT.rearrange("(k p) e -> p k e", p=P))
            nc.gpsimd.dma_start(wo_sb[:], woT.rearrange("(k p) e -> p k e", p=P))

            d2a = [psd.tile([P, SCH, NSPL], f32, name=f"d2a{i}")
                   for i in range(2)]
            # bank-wide group init: fills each d2 bank with -inv2v*c2 (K=1)
            for i in range(2):
                nc.tensor.matmul(d2a[i][:, :, :], on1f_sb[:], cb1w_sb[:],
                                 start=True, stop=False)

            prev = None  # software pipelining: d2(e-1) emitted after qps(e)
            for e in range(KC):
                qps = [psq.tile([P, 512], f32, tag="qps", name=f"qps{e}_{i}")
                       for i in range(4)]
                for k in range(KC):
                    for s4 in range(4):
                        nc.tensor.matmul(
                            qps[s4], wq[:, k, ts(e, P)], xq[:, k, ts(s4, 512)],
                            start=(k == 0), stop=(k == KC - 1))
                if prev is not None:
                    qep_prev, sqp_prev, eprev = prev
                    for tb in range(TCH):
                        sl = d2a[tb // SCH][:, tb % SCH]
                        nc.tensor.matmul(sl, qep_prev[:, ts(tb, P)],
                                         cts_sb[:, eprev], start=False,
                                         stop=False)
                        nc.tensor.matmul(sl, sqp_prev[:, ts(tb, P)],
                                         o64s_sb[:], start=False, stop=False)
                qe = qep.tile([P, S], bf16, tag="qe")
                for s4 in range(4):
                    if s4 % 2 == 0:
                        nc.scalar.activation(qe[:, ts(s4, 512)], qps[s4],
                                             AF.Identity, bias=bq_sb[:, ds(e, 1)])
                    else:
                        nc.vector.tensor_scalar_add(qe[:, ts(s4, 512)], qps[s4],
                                                    bq_sb[:, ds(e, 1)])
                sq = sqp.tile([P, S], bf16, tag="sq")
                nc.vector.tensor_mul(sq, qe, qe)
                prev = (qe, sq, e)
            qe, sq, e = prev
            # own-rows tile (tb 0..7) finishes first so its exp can start
            # while tb 8..15 still accumulate
            for tb in range(TCH):
                sl = d2a[tb // SCH][:, tb % SCH]
                nc.tensor.matmul(sl, qe[:, ts(tb, P)], cts_sb[:, e],
                                 start=False, stop=False)
                nc.tensor.matmul(sl, sq[:, ts(tb, P)], o64s_sb[:],
                                 start=False, stop=(tb % SCH == SCH - 1))
                if tb == SCH - 1:
                    nc.scalar.activation(gts[:, 0:SCH], d2a[0][:], AF.Exp)
            nc.scalar.activation(gts[:, SCH:TCH], d2a[1][:], AF.Exp)

        # ---------------- Phase B: gsum, G^T, rs, H chain ----------------
        # amp is folded into M's rows and into gsum (U = G @ diag(amp) @ M),
        # so G^T transposes run straight off the exps with no amp multiply.
        with tc.tile_pool(name="gat", bufs=2, space="PSUM") as gat, \
             tc.tile_pool(name="gsp", bufs=1, space="PSUM") as gsp, \
             tc.tile_pool(name="rsp", bufs=1, space="PSUM") as rsp, \
             tc.tile_pool(name="hrt", bufs=2, space="PSUM") as hrt:
            nc.gpsimd.memset(gaT[NSPL:NSPL + 2], 1.0)
            for sc in range(SCH):
                gatp = gat.tile([NSPL, P], bf16, tag="gat")
                nc.tensor.transpose(gatp, gts[:, sc], id_sb[:])
                if sc % 2 == 0:
                    nc.vector.tensor_copy(gaT[0:NSPL, sc], gatp)
                else:
                    nc.scalar.activation(gaT[0:NSPL, sc], gatp, AF.Copy)
            gsps = gsp.tile([1, NSPL], f32, name="gsps")
            for tb in range(TCH):
                nc.tensor.matmul(gsps, oncl_sb[:], gts[:, tb],
                                 start=(tb == 0), stop=(tb == TCH - 1))
            nc.scalar.activation(gsum_sb[:], gsps, AF.Copy)
            gscps = gsp.tile([NSPL, 1], f32, name="gscps")
            nc.tensor.matmul(gscps, gsum_sb[:], o11f_sb[:], start=True,
                             stop=True)
            nc.vector.tensor_copy(gsumc_sb[:], gscps)
            nc.vector.tensor_mul(gsa_sb[:], gsumc_sb[:], ampc_sb)
            nc.gpsimd.memset(gse_sb[:], 0.0)
            nc.vector.tensor_copy(gse_sb[0:NSPL], gsa_sb[:])
            # rs = G @ (amp*gsum) (+eps via init matmul); hoisted off the tail
            rsps = rsp.tile([P, SCH], f32, name="rsps")
            nc.tensor.matmul(rsps, on1b_sb[:], epsr_sb[:], start=True,
                             stop=False)
            for sc in range(SCH):
                nc.tensor.matmul(rsps[:, ds(sc, 1)], gaT[:, sc], gse_sb[:],
                                 start=False, stop=(sc == SCH - 1))
            nc.vector.tensor_copy(rs_sb[:], rsps)
            nc.vector.reciprocal(rcp_sb[:], rs_sb[:])
            # HrawT[e,n] = sum_t value[t,e] G[t,n]; 4 e-chunks per psum bank
            for g in range(2):
                h = hrt.tile([P, 4, NSPL], f32, tag="hrt")
                nc.tensor.matmul(h[:, :, :], on1b_sb[:], zrow_sb[:, 0:256],
                                 start=True, stop=False)
                for i in range(4):
                    e = g * 4 + i
                    for t in range(TCH):
                        nc.tensor.matmul(h[:, i], vrl_sb[:, t, ts(e, P)],
                                         gts[:, t], start=False,
                                         stop=(i == 3 and t == TCH - 1))
                if g == 0:
                    nc.scalar.activation(hrawT_sb[:, 0:4], h, AF.Copy)
                else:
                    nc.vector.tensor_copy(hrawT_sb[:, 4:KC], h)

        # ---------------- Phase C: HT, M, U, y ----------------
        yr = y.rearrange("(c p) e -> c p e", p=P)
        with tc.tile_pool(name="mp", bufs=1, space="PSUM") as mp:
            mps = [mp.tile([NSPL, 512], f32, name=f"mps{j}") for j in range(2)]
            with tc.tile_pool(name="htp", bufs=2, space="PSUM") as htp:
                # HT[e',n] = sum_d Wv[e',d] HrawT[d,n]; 4 e'-chunks per bank
                for g in range(2):
                    h2 = htp.tile([P, 4, NSPL], f32, tag="htp")
                    nc.tensor.matmul(h2[:, :, :], on1b_sb[:],
                                     zrow_sb[:, 0:256], start=True, stop=False)
                    for i in range(4):
                        ec = g * 4 + i
                        for d in range(KC):
                            nc.tensor.matmul(h2[:, i], wv_sb[:, d, ts(ec, P)],
                                             hrawT_sb[:, d], start=False,
                                             stop=(i == 3 and d == KC - 1))
                    if g == 0:
                        nc.scalar.activation(ht_sb[:, 0:4], h2, AF.Copy)
                    else:
                        nc.vector.tensor_copy(ht_sb[:, 4:KC], h2)
                    # M low half accumulates as HT chunks land
                    for i in range(4):
                        ec = g * 4 + i
                        nc.tensor.matmul(mps[0], ht_sb[:, ec],
                                         wo_sb[:, ec, ts(0, 512)],
                                         start=(ec == 0), stop=(ec == KC - 1))
            nc.vector.tensor_scalar_mul(t1_sb[:], w1b_sb[:, 0:EMBED],
                                        gsa_sb[:])
            nc.vector.affine_then_add(m_sb[0:NSPL, ts(0, 512)], mps[0],
                                      t1_sb[:, ts(0, 512)], ampc_sb, 0.0)
            with tc.tile_pool(name="ups", bufs=6, space="PSUM") as ups, \
                 tc.tile_pool(name="yb", bufs=6) as yb:
                for ec in range(KC):
                    nc.tensor.matmul(mps[1], ht_sb[:, ec],
                                     wo_sb[:, ec, ts(1, 512)],
                                     start=(ec == 0), stop=(ec == KC - 1))
                # ua (low half of y) overlaps with the mps[1] accumulation
                uas = []
                for sc in range(SCH):
                    ua = ups.tile([P, 512], f32, tag="ups", name=f"ua{sc}")
                    nc.tensor.matmul(ua, gaT[:, sc], m_sb[:, ts(0, 512)],
                                     start=True, stop=True)
                    uas.append(ua)
                nc.vector.affine_then_add(m_sb[0:NSPL, ts(1, 512)], mps[1],
                                          t1_sb[:, ts(1, 512)], ampc_sb, 0.0)
                for sc in range(SCH):
                    ub = ups.tile([P, 512], f32, tag="ups", name=f"ub{sc}")
                    nc.tensor.matmul(ub, gaT[:, sc], m_sb[:, ts(1, 512)],
                                     start=True, stop=True)
                    ysb = yb.tile([P, EMBED], bf16, tag="ysb")
                    nc.scalar.activation(ysb[:, ts(0, 512)], uas[sc], AF.Copy,
                                         scale=rcp_sb[:, ds(sc, 1)])
                    nc.vector.tensor_scalar_mul(ysb[:, ts(1, 512)], ub,
                                                rcp_sb[:, ds(sc, 1)])
                    ydma = nc.sync if sc % 2 == 0 else nc.gpsimd
                    ydma.dma_start(yr[sc], ysb)
        cpool_cm.__exit__(None, None, None)

    nc.finalize()
    return nc


def _prep_inputs(query, key, value, Wq, bq, Wk, bk, Wv, bv, Wo, bo,
                 splat_centers, splat_log_scales, splat_amplitudes):
    """Build the 8 per-core input maps (host-side sharding/layout prep)."""
    f = np.float32
    q = np.asarray(query, f)
    v = np.asarray(value, f)
    Wq = np.asarray(Wq, f); bq = np.asarray(bq, f)
    Wv = np.asarray(Wv, f); bv = np.asarray(bv, f)
    Wo = np.asarray(Wo, f); bo = np.asarray(bo, f)
    C = np.asarray(splat_centers, f)
    ls = np.asarray(splat_log_scales, f)
    amp = np.asarray(splat_amplitudes, f)

    wqT = np.ascontiguousarray(Wq.T).astype(BF16)
    wvT = np.ascontiguousarray(Wv.T).astype(BF16)
    woT = np.ascontiguousarray(Wo.T).astype(BF16)
    bq2 = np.ascontiguousarray(bq.reshape(KC, P).T)
    inv2v = (0.5 * np.exp(-2.0 * ls)).astype(np.float64)
    c2 = (C.astype(np.float64) ** 2).sum(1)
    # exponent arg = -inv2v*d2 = (2*inv2v)*q.c + (-inv2v)*|q|^2 + (-inv2v*c2)
    cts = np.ascontiguousarray((2.0 * inv2v[:, None] * C).T).astype(BF16)
    w1 = (Wo.astype(np.float64) @ bv.astype(np.float64) + bo).astype(f)
    w1b = np.empty((NSPL, EMBED + 1), f)
    w1b[:, 0:EMBED] = w1[None, :]
    w1b[:, EMBED] = amp.astype(f)
    # eps*bo as bf16 hi + lo (residual) rows: ~16 mantissa bits combined
    ebo = (EPS * bo).astype(f)
    ehi = ebo.astype(BF16)
    elo = (ebo - ehi.astype(f)).astype(BF16)
    epsbo = np.ascontiguousarray(np.stack([ehi, elo]))

    # blob_b [128, 193] bf16: o64s(64) | id128(128) | onecol(1)
    blob_b = np.empty((P, 193), BF16)
    blob_b[:, 0:NSPL] = (-inv2v).astype(f)[None, :]
    blob_b[:, NSPL:NSPL + P] = np.eye(P, dtype=BF16)
    blob_b[:, 192] = 1.0
    # blob1b [1, 648] bf16: ones(128) | epsrow(8) | zeros(512)
    blob1b = np.zeros((1, 648), BF16)
    blob1b[0, 0:P] = 1.0
    blob1b[0, P:P + SCH] = EPS
    # blob1f [1, 641] f32: cb1w(512) | ones(128) | one(1)
    blob1f = np.empty((1, 641), f)
    blob1f[0, 0:512] = np.tile((-inv2v * c2).astype(f), SCH)
    blob1f[0, 512:641] = 1.0

    shared = dict(wqT=wqT, wvT=wvT, woT=woT, bq2=bq2, cts=cts,
                  blob_b=blob_b, blob1b=blob1b, blob1f=blob1f,
                  w1b=w1b, epsbo=epsbo)
    in_maps = []
    for c in range(NCORES):
        b, h = c // 2, c % 2
        # roll the sequence axis so own rows are always 0..1023
        qb = np.concatenate([q[b, h * SOWN:], q[b, :h * SOWN]], axis=0)
        vb = np.concatenate([v[b, h * SOWN:], v[b, :h * SOWN]], axis=0)
        m = dict(shared)
        m["xqT"] = np.ascontiguousarray(qb.T).astype(BF16)
        m["vrl"] = np.ascontiguousarray(vb).astype(BF16)
        in_maps.append(m)
    return in_maps


def run_cores(inputs, trace=False):
    """Run the SPMD kernel; returns (full_output, BassKernelResults)."""
    global _PROG
    from concourse.bass_utils import run_bass_kernel_spmd
    if _PROG is None:
        _PROG = _build_program()
    nc = _PROG
    in_maps = _prep_inputs(**inputs)
    res = run_bass_kernel_spmd(nc, in_maps, list(range(NCORES)), trace=trace)
    out = np.empty((B, S, EMBED), np.float32)
    for c in range(NCORES):
        b, h = c // 2, c % 2
        out[b, h * SOWN:(h + 1) * SOWN] = res.results[c]["y"].astype(np.float32)
    return out, res


def kernel(**inputs):
    out, _ = run_cores(inputs, trace=False)
    return out


# revision 40
# speedup vs baseline: 3.3265x; 1.0338x over previous
"""HSA (hierarchical splat attention) Bass kernel for Trainium2, 8 NeuronCores.

Math (per batch b):
    q = query @ Wq.T + bq                      [S, D]
    d2[s,n]  = |q_s|^2 - 2 q_s.c_n + |c_n|^2
    G[s,n]   = exp(-d2[s,n] * inv2v[n]),  inv2v = 0.5*exp(-2*log_scales)
    A        = (G diag(amp) G^T) row-normalized (+eps)
    out      = A @ (value @ Wv.T + bv) ;  y = out @ Wo.T + bo

Because A = G diag(a) G^T is rank-64, A is never materialized:
    gsum[n]  = sum_t G[t,n]
    Hraw     = G^T @ value                       [N, D]
    M        = Hraw @ Wv.T @ Wo.T + gsum (x) w1  [N, D],  w1 = Wo@bv + bo
    rs[s]    = (amp*G)[s,:] @ gsum  (+ eps)
    y[s,:]   = ((amp*G)[s,:] @ M + eps*bo) / rs[s]
The eps*bo term makes the G-underflow case exact: rs=eps, y=bo.
bo and eps ride inside the matmuls via an appended ones-row in Ga
(row 64) matching an eps*bo row in M.

Sharding: 8 cores = (batch b = c//2, seq-half h = c%2), no collectives.
Each core computes full-batch q-proj/G (needed for gsum/Hraw) and its own
1024 output rows. The sequence axis is rolled per-core so own rows are
always t-chunks 0..7 (valid: the t-contractions are permutation-invariant).

Device dataflow (matmul = lhsT.T @ rhs, contraction on partitions):
  qT[e,s]    : lhsT=wq chunk, rhs=xq chunk          (accum over d)
  d2t[t,n]   : psum [128,8,64] x2; bank-wide K=1 init matmul injects
               -inv2v*c2 (start=True), then per e: lhsT=qe[:,tb] rhs=cts
               (+2*inv2v*q.c) and lhsT=sq[:,tb] rhs=o64s (-inv2v*|q|^2)
  G,Ga       : ACT exp (one [128,512] op per tile); Ga = G * ampw (DVE)
  gsum       : lhsT=ones col, rhs=G t-chunks -> [1,64]; PE-transpose -> [64,1]
  HrawT[e,n] : lhsT=vrl[t, e-chunk], rhs=G[t] t-chunk   (accum over t)
  HT[e',n]   : lhsT=wvT chunk,  rhs=HrawT chunk         (accum over d)
  M[n,e']    : lhsT=HT chunk,   rhs=woT chunk           (accum over e')
               + rank-1 gsum (x) w1 via DVE; -> m_sb [65,1024] (row64=eps*bo)
  rs         : psum [128,8]; K=1 init=eps; lhsT=GaT[:,sc], rhs=gsum col
  U,y        : lhsT=GaT[:,sc] [65,128], rhs=m_sb -> U; y = U * recip(rs)
"""

import numpy as np
import ml_dtypes

BF16 = ml_dtypes.bfloat16
EMBED = 1024
S = 2048
NSPL = 64
B = 4
NCORES = 8
P = 128
KC = EMBED // P   # 8 contraction chunks over d/e
TCH = S // P      # 16 t-chunks
SOWN = S // 2     # 1024 own output rows per core
SCH = SOWN // P   # 8
EPS = 1e-8

_PROG = None  # cached program


def _build_program():
    import concourse.bass as bass
    import concourse.mybir as mybir
    from concourse import bacc
    from concourse.tile import TileContext
    from concourse.bass import ts, ds

    f32 = mybir.dt.float32
    bf16 = mybir.dt.bfloat16
    AF = mybir.ActivationFunctionType

    nc = bacc.Bacc("TRN2", target_bir_lowering=False, debug=False)
    xqT = nc.declare_dram_parameter("xqT", [EMBED, S], bf16, isOutput=False)
    vrl = nc.declare_dram_parameter("vrl", [S, EMBED], bf16, isOutput=False)
    wqT = nc.declare_dram_parameter("wqT", [EMBED, EMBED], bf16, isOutput=False)
    wvT = nc.declare_dram_parameter("wvT", [EMBED, EMBED], bf16, isOutput=False)
    woT = nc.declare_dram_parameter("woT", [EMBED, EMBED], bf16, isOutput=False)
    cts = nc.declare_dram_parameter("cts", [EMBED, NSPL], bf16, isOutput=False)
    bq2 = nc.declare_dram_parameter("bq2", [P, KC], f32, isOutput=False)
    # packed constants: fewer DMA instructions (HWDGE serializes per-DMA)
    # blob_b [128, 193] bf16: o64s(64) | id128(128) | onecol(1)
    blob_b = nc.declare_dram_parameter("blob_b", [P, 193], bf16, isOutput=False)
    # blob1b [1, 648] bf16: ones(128) | epsrow(8) | zeros(512)
    blob1b = nc.declare_dram_parameter("blob1b", [1, 648], bf16, isOutput=False)
    # blob1f [1, 641] f32: cb1w(512) | ones(128) | one(1)
    blob1f = nc.declare_dram_parameter("blob1f", [1, 641], f32, isOutput=False)
    # w1b [64, 1025] f32: broadcast (Wo@bv + bo) | amp column
    w1b = nc.declare_dram_parameter("w1b", [NSPL, EMBED + 1], f32,
                                    isOutput=False)
    # eps*bo split hi/lo so the bf16 rank-1 rows carry ~16 mantissa bits
    epsbo = nc.declare_dram_parameter("epsbo", [2, EMBED], bf16, isOutput=False)
    y = nc.declare_dram_parameter("y", [SOWN, EMBED], bf16, isOutput=True)

    with TileContext(nc) as tc:
        cpool_cm = tc.tile_pool(name="const", bufs=1)
        cpool = cpool_cm.__enter__()
        bq_sb = cpool.tile([P, KC], f32)
        cts_sb = cpool.tile([P, KC, NSPL], bf16)
        bb_sb = cpool.tile([P, 193], bf16)
        b1b_sb = cpool.tile([1, 648], bf16)
        b1f_sb = cpool.tile([1, 641], f32)
        w1b_sb = cpool.tile([NSPL, EMBED + 1], f32)
        sqacc = cpool.tile([P, S], bf16)           # sum of qe^2 over e-chunks
        gts = cpool.tile([P, TCH, NSPL], bf16)     # G in [t, n] layout
        gaT = cpool.tile([NSPL + 2, SCH, P], bf16)  # G^T own rows + ones rows
        vrl_sb = cpool.tile([P, TCH, EMBED], bf16)
        wv_sb = cpool.tile([P, KC, EMBED], bf16)
        wo_sb = cpool.tile([P, KC, EMBED], bf16)
        hrawT_sb = cpool.tile([P, KC, NSPL], bf16)
        ht_sb = cpool.tile([P, KC, NSPL], bf16)
        m_sb = cpool.tile([NSPL + 2, EMBED], bf16)
        t1_sb = cpool.tile([NSPL, EMBED], f32)
        gsum_sb = cpool.tile([1, NSPL], f32)
        gsumc_sb = cpool.tile([NSPL, 1], f32)
        gsa_sb = cpool.tile([NSPL, 1], f32)        # amp * gsum
        gse_sb = cpool.tile([NSPL + 2, 1], bf16)
        rs_sb = cpool.tile([P, SCH], f32)
        rcp_sb = cpool.tile([P, SCH], f32)

        # const views into packed blobs
        o64s_sb = bb_sb[:, 0:NSPL]
        id_sb = bb_sb[:, NSPL:NSPL + P]
        oncl_sb = bb_sb[:, 192:193]
        on1b_sb = b1b_sb[:, 0:P]
        epsr_sb = b1b_sb[:, P:P + SCH]
        zrow_sb = b1b_sb[:, 136:648]
        cb1w_sb = b1f_sb[:, 0:512]
        on1f_sb = b1f_sb[:, 512:512 + P]
        o11f_sb = b1f_sb[:, 640:641]
        ampc_sb = w1b_sb[:, EMBED:EMBED + 1]

        # ---------------- Phase A: q projection + d2 in [t, n] ----------------
        with tc.tile_pool(name="pa", bufs=1) as pa, \
             tc.tile_pool(name="qe", bufs=2) as qep, \
             tc.tile_pool(name="sqe", bufs=2) as sqp, \
             tc.tile_pool(name="psq", bufs=4, space="PSUM") as psq, \
             tc.tile_pool(name="psd", bufs=1, space="PSUM") as psd:
            xq = pa.tile([P, KC, S], bf16)
            wq = pa.tile([P, KC, EMBED], bf16)
            wqr = wqT.rearrange("(k p) e -> k p e", p=P)
            xqr = xqT.rearrange("(k p) s -> k p s", p=P)
            # critical-path chunks spread over the SP/Activation/Pool queues
            # (v1 charges transfer time to the issuing engine); k=0 split
            # into small pieces so the first matmuls start ASAP
            nc.sync.dma_start(wq[:, 0, 0:512], wqr[0][:, 0:512])
            nc.gpsimd.dma_start(xq[:, 0, 0:512], xqr[0][:, 0:512])
            nc.sync.dma_start(wq[:, 0, 512:EMBED], wqr[0][:, 512:EMBED])
            nc.sync.dma_start(xq[:, 0, 512:S], xqr[0][:, 512:S])
            # b1f early: the d2 psum-init matmuls read cb1w from it
            nc.gpsimd.dma_start(b1f_sb[:], blob1f[:])
            qeng = {1: nc.scalar, 2: nc.gpsimd, 3: nc.sync, 4: nc.scalar,
                    5: nc.gpsimd, 6: nc.sync, 7: nc.scalar}
            for k in range(1, KC):
                qeng[k].dma_start(wq[:, k], wqr[k])
                qeng[k].dma_start(xq[:, k], xqr[k])
            # remaining constants + bulk prefetch on the Pool engine
            nc.gpsimd.dma_start(bq_sb[:], bq2[:])
            nc.gpsimd.dma_start(cts_sb[:], cts.rearrange("(k p) n -> p k n", p=P))
            nc.gpsimd.dma_start(bb_sb[:], blob_b[:])
            nc.gpsimd.dma_start(b1b_sb[:], blob1b[:])
            nc.gpsimd.dma_start(m_sb[NSPL:NSPL + 2, :], epsbo[:])
            nc.gpsimd.dma_start(w1b_sb[:], w1b[:])
            nc.gpsimd.dma_start(vrl_sb[:], vrl.rearrange("(t p) e -> p t e", p=P))
            nc.gpsimd.dma_start(wv_sb[:], wvT.rearrange("(k p) e -> p k e", p=P))
            nc.gpsimd.dma_start(wo_sb[:], woT.rearrange("(k p) e -> p k e", p=P))

            d2a = [psd.tile([P, 4, NSPL], f32, name=f"d2a{i}")
                   for i in range(4)]
            # bank-wide group init: fills each d2 bank with -inv2v*c2 (K=1)
            for i in range(4):
                nc.tensor.matmul(d2a[i][:, :, :], on1f_sb[:],
                                 cb1w_sb[:, 0:256], start=True, stop=False)

            prev = None  # software pipelining: d2(e-1) emitted after qps(e)
            for e in range(KC):
                qps = [psq.tile([P, 512], f32, tag="qps", name=f"qps{e}_{i}")
                       for i in range(4)]
                for k in range(KC):
                    for s4 in range(4):
                        nc.tensor.matmul(
                            qps[s4], wq[:, k, ts(e, P)], xq[:, k, ts(s4, 512)],
                            start=(k == 0), stop=(k == KC - 1))
                if prev is not None:
                    qep_prev, eprev = prev
                    # ct-term only; |q|^2 accumulates on DVE into sqacc
                    for tb in range(TCH):
                        sl = d2a[tb // 4][:, tb % 4]
                        nc.tensor.matmul(sl, qep_prev[:, ts(tb, P)],
                                         cts_sb[:, eprev], start=False,
                                         stop=False)
                qe = qep.tile([P, S], bf16, tag="qe")
                for s4 in range(4):
                    if s4 % 2 == 0:
                        nc.scalar.activation(qe[:, ts(s4, 512)], qps[s4],
                                             AF.Identity, bias=bq_sb[:, ds(e, 1)])
                    else:
                        nc.vector.tensor_scalar_add(qe[:, ts(s4, 512)], qps[s4],
                                                    bq_sb[:, ds(e, 1)])
                if e < KC - 1:
                    if e == 0:
                        nc.vector.tensor_mul(sqacc[:], qe, qe)
                    else:
                        sq = sqp.tile([P, S], bf16, tag="sq")
                        nc.vector.tensor_mul(sq, qe, qe)
                        nc.vector.tensor_add(sqacc[:], sqacc[:], sq)
                else:
                    sq7 = sqp.tile([P, S], bf16, tag="sq")
                    nc.vector.tensor_mul(sq7, qe, qe)
                prev = (qe, e)
            qe, e = prev
            # each quarter-tile stops early so its exp overlaps the rest
            for tb in range(TCH):
                sl = d2a[tb // 4][:, tb % 4]
                nc.tensor.matmul(sl, qe[:, ts(tb, P)], cts_sb[:, e],
                                 start=False, stop=False)
                nc.tensor.matmul(sl, sqacc[:, ts(tb, P)], o64s_sb[:],
                                 start=False, stop=False)
                nc.tensor.matmul(sl, sq7[:, ts(tb, P)], o64s_sb[:],
                                 start=False, stop=(tb % 4 == 3))
                if tb % 4 == 3:
                    nc.scalar.activation(gts[:, tb - 3:tb + 1],
                                         d2a[tb // 4][:], AF.Exp)

        # ---------------- Phase B: gsum, G^T, rs, H chain ----------------
        # amp is folded into M's rows and into gsum (U = G @ diag(amp) @ M),
        # so G^T transposes run straight off the exps with no amp multiply.
        with tc.tile_pool(name="gat", bufs=2, space="PSUM") as gat, \
             tc.tile_pool(name="gsp", bufs=1, space="PSUM") as gsp, \
             tc.tile_pool(name="rsp", bufs=1, space="PSUM") as rsp, \
             tc.tile_pool(name="hrt", bufs=2, space="PSUM") as hrt:
            nc.gpsimd.memset(gaT[NSPL:NSPL + 2], 1.0)
            for sc in range(SCH):
                gatp = gat.tile([NSPL, P], bf16, tag="gat")
                nc.tensor.transpose(gatp, gts[:, sc], id_sb[:])
                if sc % 2 == 0:
                    nc.vector.tensor_copy(gaT[0:NSPL, sc], gatp)
                else:
                    nc.scalar.activation(gaT[0:NSPL, sc], gatp, AF.Copy)
            gsps = gsp.tile([1, NSPL], f32, name="gsps")
            for tb in range(TCH):
                nc.tensor.matmul(gsps, oncl_sb[:], gts[:, tb],
                                 start=(tb == 0), stop=(tb == TCH - 1))
            nc.scalar.activation(gsum_sb[:], gsps, AF.Copy)
            gscps = gsp.tile([NSPL, 1], f32, name="gscps")
            nc.tensor.matmul(gscps, gsum_sb[:], o11f_sb[:], start=True,
                             stop=True)
            nc.vector.tensor_copy(gsumc_sb[:], gscps)
            nc.vector.tensor_mul(gsa_sb[:], gsumc_sb[:], ampc_sb)
            nc.gpsimd.memset(gse_sb[:], 0.0)
            nc.vector.tensor_copy(gse_sb[0:NSPL], gsa_sb[:])
            # rs = G @ (amp*gsum) (+eps via init matmul); hoisted off the tail
            rsps = rsp.tile([P, SCH], f32, name="rsps")
            nc.tensor.matmul(rsps, on1b_sb[:], epsr_sb[:], start=True,
                             stop=False)
            for sc in range(SCH):
                nc.tensor.matmul(rsps[:, ds(sc, 1)], gaT[:, sc], gse_sb[:],
                                 start=False, stop=(sc == SCH - 1))
            nc.vector.tensor_copy(rs_sb[:], rsps)
            nc.vector.reciprocal(rcp_sb[:], rs_sb[:])
            # HrawT[e,n] = sum_t value[t,e] G[t,n]; 4 e-chunks per psum bank
            for g in range(2):
                h = hrt.tile([P, 4, NSPL], f32, tag="hrt")
                nc.tensor.matmul(h[:, :, :], on1b_sb[:], zrow_sb[:, 0:256],
                                 start=True, stop=False)
                for i in range(4):
                    e = g * 4 + i
                    for t in range(TCH):
                        nc.tensor.matmul(h[:, i], vrl_sb[:, t, ts(e, P)],
                                         gts[:, t], start=False,
                                         stop=(i == 3 and t == TCH - 1))
                if g == 0:
                    nc.scalar.activation(hrawT_sb[:, 0:4], h, AF.Copy)
                else:
                    nc.vector.tensor_copy(hrawT_sb[:, 4:KC], h)

        # ---------------- Phase C: HT, M, U, y ----------------
        yr = y.rearrange("(c p) e -> c p e", p=P)
        with tc.tile_pool(name="mp", bufs=1, space="PSUM") as mp:
            mps = [mp.tile([NSPL, 512], f32, name=f"mps{j}") for j in range(2)]
            with tc.tile_pool(name="htp", bufs=2, space="PSUM") as htp:
                # HT[e',n] = sum_d Wv[e',d] HrawT[d,n]; 4 e'-chunks per bank
                for g in range(2):
                    h2 = htp.tile([P, 4, NSPL], f32, tag="htp")
                    nc.tensor.matmul(h2[:, :, :], on1b_sb[:],
                                     zrow_sb[:, 0:256], start=True, stop=False)
                    for i in range(4):
                        ec = g * 4 + i
                        for d in range(KC):
                            nc.tensor.matmul(h2[:, i], wv_sb[:, d, ts(ec, P)],
                                             hrawT_sb[:, d], start=False,
                                             stop=(i == 3 and d == KC - 1))
                    if g == 0:
                        nc.scalar.activation(ht_sb[:, 0:4], h2, AF.Copy)
                    else:
                        nc.vector.tensor_copy(ht_sb[:, 4:KC], h2)
                    # M low half accumulates as HT chunks land
                    for i in range(4):
                        ec = g * 4 + i
                        nc.tensor.matmul(mps[0], ht_sb[:, ec],
                                         wo_sb[:, ec, ts(0, 512)],
                                         start=(ec == 0), stop=(ec == KC - 1))
            nc.vector.tensor_scalar_mul(t1_sb[:], w1b_sb[:, 0:EMBED],
                                        gsa_sb[:])
            nc.vector.affine_then_add(m_sb[0:NSPL, ts(0, 512)], mps[0],
                                      t1_sb[:, ts(0, 512)], ampc_sb, 0.0)
            with tc.tile_pool(name="ups", bufs=6, space="PSUM") as ups, \
                 tc.tile_pool(name="yb", bufs=6) as yb:
                for ec in range(KC):
                    nc.tensor.matmul(mps[1], ht_sb[:, ec],
                                     wo_sb[:, ec, ts(1, 512)],
                                     start=(ec == 0), stop=(ec == KC - 1))
                # ua (low half of y) + its scale overlap the mps[1] accum;
                # scales spread over ACT/DVE/Pool so no one engine paces the
                # tail; y DMAs alternate the SP and Pool queues
                ysbs = []
                for sc in range(SCH):
                    ua = ups.tile([P, 512], f32, tag="ups", name=f"ua{sc}")
                    nc.tensor.matmul(ua, gaT[:, sc], m_sb[:, ts(0, 512)],
                                     start=True, stop=True)
                    ysb = yb.tile([P, EMBED], bf16, tag="ysb")
                    nc.scalar.activation(ysb[:, ts(0, 512)], ua, AF.Copy,
                                         scale=rcp_sb[:, ds(sc, 1)])
                    ysbs.append(ysb)
                nc.vector.affine_then_add(m_sb[0:NSPL, ts(1, 512)], mps[1],
                                          t1_sb[:, ts(1, 512)], ampc_sb, 0.0)
                for sc in range(SCH):
                    ub = ups.tile([P, 512], f32, tag="ups", name=f"ub{sc}")
                    nc.tensor.matmul(ub, gaT[:, sc], m_sb[:, ts(1, 512)],
                                     start=True, stop=True)
                    ysb = ysbs[sc]
                    nc.vector.tensor_scalar_mul(ysb[:, ts(1, 512)], ub,
                                                rcp_sb[:, ds(sc, 1)])
                    ydma = nc.sync if sc % 2 == 0 else nc.gpsimd
                    ydma.dma_start(yr[sc], ysb)
        cpool_cm.__exit__(None, None, None)

    nc.finalize()
    return nc


def _prep_inputs(query, key, value, Wq, bq, Wk, bk, Wv, bv, Wo, bo,
                 splat_centers, splat_log_scales, splat_amplitudes):
    """Build the 8 per-core input maps (host-side sharding/layout prep)."""
    f = np.float32
    q = np.asarray(query, f)
    v = np.asarray(value, f)
    Wq = np.asarray(Wq, f); bq = np.asarray(bq, f)
    Wv = np.asarray(Wv, f); bv = np.asarray(bv, f)
    Wo = np.asarray(Wo, f); bo = np.asarray(bo, f)
    C = np.asarray(splat_centers, f)
    ls = np.asarray(splat_log_scales, f)
    amp = np.asarray(splat_amplitudes, f)

    wqT = np.ascontiguousarray(Wq.T).astype(BF16)
    wvT = np.ascontiguousarray(Wv.T).astype(BF16)
    woT = np.ascontiguousarray(Wo.T).astype(BF16)
    bq2 = np.ascontiguousarray(bq.reshape(KC, P).T)
    inv2v = (0.5 * np.exp(-2.0 * ls)).astype(np.float64)
    c2 = (C.astype(np.float64) ** 2).sum(1)
    # exponent arg = -inv2v*d2 = (2*inv2v)*q.c + (-inv2v)*|q|^2 + (-inv2v*c2)
    cts = np.ascontiguousarray((2.0 * inv2v[:, None] * C).T).astype(BF16)
    w1 = (Wo.astype(np.float64) @ bv.astype(np.float64) + bo).astype(f)
    w1b = np.empty((NSPL, EMBED + 1), f)
    w1b[:, 0:EMBED] = w1[None, :]
    w1b[:, EMBED] = amp.astype(f)
    # eps*bo as bf16 hi + lo (residual) rows: ~16 mantissa bits combined
    ebo = (EPS * bo).astype(f)
    ehi = ebo.astype(BF16)
    elo = (ebo - ehi.astype(f)).astype(BF16)
    epsbo = np.ascontiguousarray(np.stack([ehi, elo]))

    # blob_b [128, 193] bf16: o64s(64) | id128(128) | onecol(1)
    blob_b = np.empty((P, 193), BF16)
    blob_b[:, 0:NSPL] = (-inv2v).astype(f)[None, :]
    blob_b[:, NSPL:NSPL + P] = np.eye(P, dtype=BF16)
    blob_b[:, 192] = 1.0
    # blob1b [1, 648] bf16: ones(128) | epsrow(8) | zeros(512)
    blob1b = np.zeros((1, 648), BF16)
    blob1b[0, 0:P] = 1.0
    blob1b[0, P:P + SCH] = EPS
    # blob1f [1, 641] f32: cb1w(512) | ones(128) | one(1)
    blob1f = np.empty((1, 641), f)
    blob1f[0, 0:512] = np.tile((-inv2v * c2).astype(f), SCH)
    blob1f[0, 512:641] = 1.0

    shared = dict(wqT=wqT, wvT=wvT, woT=woT, bq2=bq2, cts=cts,
                  blob_b=blob_b, blob1b=blob1b, blob1f=blob1f,
                  w1b=w1b, epsbo=epsbo)
    in_maps = []
    for c in range(NCORES):
        b, h = c // 2, c % 2
        # roll the sequence axis so own rows are always 0..1023
        qb = np.concatenate([q[b, h * SOWN:], q[b, :h * SOWN]], axis=0)
        vb = np.concatenate([v[b, h * SOWN:], v[b, :h * SOWN]], axis=0)
        m = dict(shared)
        m["xqT"] = np.ascontiguousarray(qb.T).astype(BF16)
        m["vrl"] = np.ascontiguousarray(vb).astype(BF16)
        in_maps.append(m)
    return in_maps


def run_cores(inputs, trace=False):
    """Run the SPMD kernel; returns (full_output, BassKernelResults)."""
    global _PROG
    from concourse.bass_utils import run_bass_kernel_spmd
    if _PROG is None:
        _PROG = _build_program()
    nc = _PROG
    in_maps = _prep_inputs(**inputs)
    res = run_bass_kernel_spmd(nc, in_maps, list(range(NCORES)), trace=trace)
    out = np.empty((B, S, EMBED), np.float32)
    for c in range(NCORES):
        b, h = c // 2, c % 2
        out[b, h * SOWN:(h + 1) * SOWN] = res.results[c]["y"].astype(np.float32)
    return out, res


def kernel(**inputs):
    out, _ = run_cores(inputs, trace=False)
    return out


# revision 46
# speedup vs baseline: 3.4263x; 1.0300x over previous
"""HSA (hierarchical splat attention) Bass kernel for Trainium2, 8 NeuronCores.

Math (per batch b):
    q = query @ Wq.T + bq                      [S, D]
    d2[s,n]  = |q_s|^2 - 2 q_s.c_n + |c_n|^2
    G[s,n]   = exp(-d2[s,n] * inv2v[n]),  inv2v = 0.5*exp(-2*log_scales)
    A        = (G diag(amp) G^T) row-normalized (+eps)
    out      = A @ (value @ Wv.T + bv) ;  y = out @ Wo.T + bo

Because A = G diag(a) G^T is rank-64, A is never materialized:
    gsum[n]  = sum_t G[t,n]
    Hraw     = G^T @ value                       [N, D]
    M        = Hraw @ Wv.T @ Wo.T + gsum (x) w1  [N, D],  w1 = Wo@bv + bo
    rs[s]    = (amp*G)[s,:] @ gsum  (+ eps)
    y[s,:]   = ((amp*G)[s,:] @ M + eps*bo) / rs[s]
The eps*bo term makes the G-underflow case exact: rs=eps, y=bo.
bo and eps ride inside the matmuls via an appended ones-row in Ga
(row 64) matching an eps*bo row in M.

Sharding: 8 cores = (batch b = c//2, seq-half h = c%2), no collectives.
Each core computes full-batch q-proj/G (needed for gsum/Hraw) and its own
1024 output rows. The sequence axis is rolled per-core so own rows are
always t-chunks 0..7 (valid: the t-contractions are permutation-invariant).

Device dataflow (matmul = lhsT.T @ rhs, contraction on partitions):
  qT[e,s]    : lhsT=wq chunk, rhs=xq chunk          (accum over d)
  d2t[t,n]   : psum [128,8,64] x2; bank-wide K=1 init matmul injects
               -inv2v*c2 (start=True), then per e: lhsT=qe[:,tb] rhs=cts
               (+2*inv2v*q.c) and lhsT=sq[:,tb] rhs=o64s (-inv2v*|q|^2)
  G,Ga       : ACT exp (one [128,512] op per tile); Ga = G * ampw (DVE)
  gsum       : lhsT=ones col, rhs=G t-chunks -> [1,64]; PE-transpose -> [64,1]
  HrawT[e,n] : lhsT=vrl[t, e-chunk], rhs=G[t] t-chunk   (accum over t)
  HT[e',n]   : lhsT=wvT chunk,  rhs=HrawT chunk         (accum over d)
  M[n,e']    : lhsT=HT chunk,   rhs=woT chunk           (accum over e')
               + rank-1 gsum (x) w1 via DVE; -> m_sb [65,1024] (row64=eps*bo)
  rs         : psum [128,8]; K=1 init=eps; lhsT=GaT[:,sc], rhs=gsum col
  U,y        : lhsT=GaT[:,sc] [65,128], rhs=m_sb -> U; y = U * recip(rs)
"""

import numpy as np
import ml_dtypes

BF16 = ml_dtypes.bfloat16
EMBED = 1024
S = 2048
NSPL = 64
B = 4
NCORES = 8
P = 128
KC = EMBED // P   # 8 contraction chunks over d/e
TCH = S // P      # 16 t-chunks
SOWN = S // 2     # 1024 own output rows per core
SCH = SOWN // P   # 8
EPS = 1e-8

_PROG = None  # cached program


def _build_program():
    import concourse.bass as bass
    import concourse.mybir as mybir
    from concourse import bacc
    from concourse.tile import TileContext
    from concourse.bass import ts, ds

    f32 = mybir.dt.float32
    bf16 = mybir.dt.bfloat16
    AF = mybir.ActivationFunctionType

    nc = bacc.Bacc("TRN2", target_bir_lowering=False, debug=False)
    xqT = nc.declare_dram_parameter("xqT", [EMBED, S], bf16, isOutput=False)
    vrl = nc.declare_dram_parameter("vrl", [S, EMBED], bf16, isOutput=False)
    wqT = nc.declare_dram_parameter("wqT", [EMBED, EMBED], bf16, isOutput=False)
    wvT = nc.declare_dram_parameter("wvT", [EMBED, EMBED], bf16, isOutput=False)
    woT = nc.declare_dram_parameter("woT", [EMBED, EMBED], bf16, isOutput=False)
    cts = nc.declare_dram_parameter("cts", [EMBED, NSPL], bf16, isOutput=False)
    bq2 = nc.declare_dram_parameter("bq2", [P, KC], f32, isOutput=False)
    # packed constants: fewer DMA instructions (HWDGE serializes per-DMA)
    # blob_b [128, 193] bf16: o64s(64) | id128(128) | onecol(1)
    blob_b = nc.declare_dram_parameter("blob_b", [P, 193], bf16, isOutput=False)
    # blob1b [1, 648] bf16: ones(128) | epsrow(8) | zeros(512)
    blob1b = nc.declare_dram_parameter("blob1b", [1, 648], bf16, isOutput=False)
    # blob1f [1, 641] f32: cb1w(512) | ones(128) | one(1)
    blob1f = nc.declare_dram_parameter("blob1f", [1, 641], f32, isOutput=False)
    # w1b [64, 1025] f32: broadcast (Wo@bv + bo) | amp column
    w1b = nc.declare_dram_parameter("w1b", [NSPL, EMBED + 1], f32,
                                    isOutput=False)
    # eps*bo split hi/lo so the bf16 rank-1 rows carry ~16 mantissa bits
    epsbo = nc.declare_dram_parameter("epsbo", [2, EMBED], bf16, isOutput=False)
    y = nc.declare_dram_parameter("y", [SOWN, EMBED], bf16, isOutput=True)

    with TileContext(nc) as tc:
        cpool_cm = tc.tile_pool(name="const", bufs=1)
        cpool = cpool_cm.__enter__()
        bq_sb = cpool.tile([P, KC], f32)
        cts_sb = cpool.tile([P, KC, NSPL], bf16)
        bb_sb = cpool.tile([P, 193], bf16)
        b1b_sb = cpool.tile([1, 648], bf16)
        b1f_sb = cpool.tile([1, 641], f32)
        w1b_sb = cpool.tile([NSPL, EMBED + 1], f32)
        sqacc = cpool.tile([P, S], bf16)           # sum of qe^2 over e-chunks
        gts = cpool.tile([P, TCH, NSPL], bf16)     # G in [t, n] layout
        gaT = cpool.tile([NSPL + 2, SCH, P], bf16)  # G^T own rows + ones rows
        vrl_sb = cpool.tile([P, TCH, EMBED], bf16)
        wv_sb = cpool.tile([P, KC, EMBED], bf16)
        wo_sb = cpool.tile([P, KC, EMBED], bf16)
        hrawT_sb = cpool.tile([P, KC, NSPL], bf16)
        ht_sb = cpool.tile([P, KC, NSPL], bf16)
        m_sb = cpool.tile([NSPL + 2, EMBED], bf16)
        t1_sb = cpool.tile([NSPL, EMBED], f32)
        gsum_sb = cpool.tile([1, NSPL], f32)
        gsumc_sb = cpool.tile([NSPL, 1], f32)
        gsa_sb = cpool.tile([NSPL, 1], f32)        # amp * gsum
        gse_sb = cpool.tile([NSPL + 2, 1], bf16)
        rs_sb = cpool.tile([P, SCH], f32)
        rcp_sb = cpool.tile([P, SCH], f32)

        # const views into packed blobs
        o64s_sb = bb_sb[:, 0:NSPL]
        id_sb = bb_sb[:, NSPL:NSPL + P]
        oncl_sb = bb_sb[:, 192:193]
        on1b_sb = b1b_sb[:, 0:P]
        epsr_sb = b1b_sb[:, P:P + SCH]
        zrow_sb = b1b_sb[:, 136:648]
        cb1w_sb = b1f_sb[:, 0:512]
        on1f_sb = b1f_sb[:, 512:512 + P]
        o11f_sb = b1f_sb[:, 640:641]
        ampc_sb = w1b_sb[:, EMBED:EMBED + 1]

        # ---------------- Phase A: q projection + d2 in [t, n] ----------------
        with tc.tile_pool(name="pa", bufs=1) as pa, \
             tc.tile_pool(name="qe", bufs=2) as qep, \
             tc.tile_pool(name="sqe", bufs=2) as sqp, \
             tc.tile_pool(name="psq", bufs=4, space="PSUM") as psq, \
             tc.tile_pool(name="psd", bufs=1, space="PSUM") as psd:
            xq = pa.tile([P, KC, S], bf16)
            wq = pa.tile([P, KC, EMBED], bf16)
            wqr = wqT.rearrange("(k p) e -> k p e", p=P)
            xqr = xqT.rearrange("(k p) s -> k p s", p=P)
            # critical-path chunks spread over the SP/Activation/Pool queues
            # (v1 charges transfer time to the issuing engine); k=0 split
            # into small pieces so the first matmuls start ASAP
            nc.sync.dma_start(wq[:, 0, 0:512], wqr[0][:, 0:512])
            nc.gpsimd.dma_start(xq[:, 0, 0:512], xqr[0][:, 0:512])
            nc.sync.dma_start(wq[:, 0, 512:EMBED], wqr[0][:, 512:EMBED])
            nc.sync.dma_start(xq[:, 0, 512:S], xqr[0][:, 512:S])
            # b1f early: the d2 psum-init matmuls read cb1w from it
            nc.gpsimd.dma_start(b1f_sb[:], blob1f[:])
            qeng = {1: nc.scalar, 2: nc.gpsimd, 3: nc.sync, 4: nc.scalar,
                    5: nc.gpsimd, 6: nc.sync, 7: nc.scalar}
            for k in range(1, KC):
                qeng[k].dma_start(wq[:, k], wqr[k])
                qeng[k].dma_start(xq[:, k], xqr[k])
            # remaining constants + bulk prefetch on the Pool engine
            nc.gpsimd.dma_start(bq_sb[:], bq2[:])
            nc.gpsimd.dma_start(cts_sb[:], cts.rearrange("(k p) n -> p k n", p=P))
            nc.gpsimd.dma_start(bb_sb[:], blob_b[:])
            nc.gpsimd.dma_start(b1b_sb[:], blob1b[:])
            nc.gpsimd.dma_start(m_sb[NSPL:NSPL + 2, :], epsbo[:])
            nc.gpsimd.dma_start(w1b_sb[:], w1b[:])
            nc.gpsimd.dma_start(vrl_sb[:], vrl.rearrange("(t p) e -> p t e", p=P))
            nc.gpsimd.dma_start(wv_sb[:], wvT.rearrange("(k p) e -> p k e", p=P))
            nc.gpsimd.dma_start(wo_sb[:], woT.rearrange("(k p) e -> p k e", p=P))

            d2a = [psd.tile([P, 4, NSPL], f32, name=f"d2a{i}")
                   for i in range(4)]
            # bank-wide group init: fills each d2 bank with -inv2v*c2 (K=1)
            for i in range(4):
                nc.tensor.matmul(d2a[i][:, :, :], on1f_sb[:],
                                 cb1w_sb[:, 0:256], start=True, stop=False)

            prev = None  # software pipelining: d2(e-1) emitted after qps(e)
            for e in range(KC):
                qps = [psq.tile([P, 512], f32, tag="qps", name=f"qps{e}_{i}")
                       for i in range(4)]
                for k in range(KC):
                    for s4 in range(4):
                        nc.tensor.matmul(
                            qps[s4], wq[:, k, ts(e, P)], xq[:, k, ts(s4, 512)],
                            start=(k == 0), stop=(k == KC - 1))
                if prev is not None:
                    qep_prev, eprev = prev
                    # ct-term only; |q|^2 accumulates on DVE into sqacc
                    for tb in range(TCH):
                        sl = d2a[tb // 4][:, tb % 4]
                        nc.tensor.matmul(sl, qep_prev[:, ts(tb, P)],
                                         cts_sb[:, eprev], start=False,
                                         stop=False)
                qe = qep.tile([P, S], bf16, tag="qe")
                for s4 in range(4):
                    if s4 % 2 == 0:
                        nc.scalar.activation(qe[:, ts(s4, 512)], qps[s4],
                                             AF.Identity, bias=bq_sb[:, ds(e, 1)])
                    else:
                        nc.vector.tensor_scalar_add(qe[:, ts(s4, 512)], qps[s4],
                                                    bq_sb[:, ds(e, 1)])
                if e < KC - 1:
                    if e == 0:
                        nc.vector.tensor_mul(sqacc[:], qe, qe)
                    else:
                        sq = sqp.tile([P, S], bf16, tag="sq")
                        nc.vector.tensor_mul(sq, qe, qe)
                        nc.vector.tensor_add(sqacc[:], sqacc[:], sq)
                else:
                    sq7 = sqp.tile([P, S], bf16, tag="sq")
                    nc.vector.tensor_mul(sq7[:, 0:1024], qe[:, 0:1024],
                                         qe[:, 0:1024])
                    nc.vector.tensor_mul(sq7[:, 1024:S], qe[:, 1024:S],
                                         qe[:, 1024:S])
                prev = (qe, e)
            qe, e = prev
            # ct + sqacc terms first (independent of sq7), then the sq7
            # ones-term; each quarter-tile stops early so its exp overlaps
            for tb in range(TCH):
                sl = d2a[tb // 4][:, tb % 4]
                nc.tensor.matmul(sl, qe[:, ts(tb, P)], cts_sb[:, e],
                                 start=False, stop=False)
                nc.tensor.matmul(sl, sqacc[:, ts(tb, P)], o64s_sb[:],
                                 start=False, stop=False)
            for tb in range(TCH):
                sl = d2a[tb // 4][:, tb % 4]
                nc.tensor.matmul(sl, sq7[:, ts(tb, P)], o64s_sb[:],
                                 start=False, stop=(tb % 4 == 3))
                if tb % 4 == 3:
                    nc.scalar.activation(gts[:, tb - 3:tb + 1],
                                         d2a[tb // 4][:], AF.Exp)

        # ---------------- Phase B: H chain, gsum, G^T ----------------
        # amp is folded into M's rows and into gsum (U = G @ diag(amp) @ M),
        # so G^T transposes run straight off the exps with no amp multiply.
        # PE emission order keeps the engine hot: HrawT g0 -> gsum/transposes
        # -> HrawT g1; rs waits on a DVE chain so it moves to Phase C.
        with tc.tile_pool(name="gat", bufs=2, space="PSUM") as gat, \
             tc.tile_pool(name="gsp", bufs=1, space="PSUM") as gsp, \
             tc.tile_pool(name="hrt", bufs=2, space="PSUM") as hrt:
            nc.gpsimd.memset(gaT[NSPL:NSPL + 2], 1.0)
            gsps = gsp.tile([1, NSPL], f32, name="gsps")
            gscps = gsp.tile([NSPL, 1], f32, name="gscps")
            for g in range(2):
                # HrawT[e,n] = sum_t value[t,e] G[t,n]; 4 e-chunks per bank
                h = hrt.tile([P, 4, NSPL], f32, tag="hrt")
                nc.tensor.matmul(h[:, :, :], on1b_sb[:], zrow_sb[:, 0:256],
                                 start=True, stop=False)
                for i in range(4):
                    e = g * 4 + i
                    for t in range(TCH):
                        nc.tensor.matmul(h[:, i], vrl_sb[:, t, ts(e, P)],
                                         gts[:, t], start=False,
                                         stop=(i == 3 and t == TCH - 1))
                    if g == 0 and i == 0:
                        # interleave work with matching exp-quarter deps so
                        # PE isn't gated by the serial exp stream
                        for q in range(4):
                            for tb in range(q * 4, q * 4 + 4):
                                nc.tensor.matmul(gsps, oncl_sb[:],
                                                 gts[:, tb], start=(tb == 0),
                                                 stop=(tb == TCH - 1))
                            if q < 2:
                                for sc in range(q * 4, q * 4 + 4):
                                    gatp = gat.tile([NSPL, P], bf16,
                                                    tag="gat")
                                    nc.tensor.transpose(gatp, gts[:, sc],
                                                        id_sb[:])
                                    if sc % 2 == 0:
                                        nc.vector.tensor_copy(
                                            gaT[0:NSPL, sc], gatp)
                                    else:
                                        nc.scalar.activation(
                                            gaT[0:NSPL, sc], gatp, AF.Copy)
                        nc.scalar.activation(gsum_sb[:], gsps, AF.Copy)
                        nc.tensor.matmul(gscps, gsum_sb[:], o11f_sb[:],
                                         start=True, stop=True)
                if g == 0:
                    nc.scalar.activation(hrawT_sb[:, 0:4], h, AF.Copy)
                else:
                    nc.vector.tensor_copy(hrawT_sb[:, 4:KC], h)
            nc.vector.tensor_copy(gsumc_sb[:], gscps)
            nc.vector.tensor_mul(gsa_sb[:], gsumc_sb[:], ampc_sb)
            nc.gpsimd.memset(gse_sb[:], 0.0)
            nc.vector.tensor_copy(gse_sb[0:NSPL], gsa_sb[:])

        # ---------------- Phase C: HT, M, rs, U, y ----------------
        yr = y.rearrange("(c p) e -> c p e", p=P)
        with tc.tile_pool(name="mp", bufs=1, space="PSUM") as mp:
            mps = [mp.tile([NSPL, 512], f32, name=f"mps{j}") for j in range(2)]
            with tc.tile_pool(name="htp", bufs=2, space="PSUM") as htp, \
                 tc.tile_pool(name="rsp", bufs=1, space="PSUM") as rsp:
                # HT[e',n] = sum_d Wv[e',d] HrawT[d,n]; 4 e'-chunks per bank
                for g in range(2):
                    h2 = htp.tile([P, 4, NSPL], f32, tag="htp")
                    nc.tensor.matmul(h2[:, :, :], on1b_sb[:],
                                     zrow_sb[:, 0:256], start=True, stop=False)
                    for i in range(4):
                        ec = g * 4 + i
                        for d in range(KC):
                            nc.tensor.matmul(h2[:, i], wv_sb[:, d, ts(ec, P)],
                                             hrawT_sb[:, d], start=False,
                                             stop=(i == 3 and d == KC - 1))
                    if g == 0:
                        nc.scalar.activation(ht_sb[:, 0:4], h2, AF.Copy)
                    else:
                        nc.vector.tensor_copy(ht_sb[:, 4:KC], h2)
                    # M low half accumulates as HT chunks land
                    for i in range(4):
                        ec = g * 4 + i
                        nc.tensor.matmul(mps[0], ht_sb[:, ec],
                                         wo_sb[:, ec, ts(0, 512)],
                                         start=(ec == 0), stop=(ec == KC - 1))
                # rs = G @ (amp*gsum) (+eps via init matmul)
                rsps = rsp.tile([P, SCH], f32, name="rsps")
                nc.tensor.matmul(rsps, on1b_sb[:], epsr_sb[:], start=True,
                                 stop=False)
                for sc in range(SCH):
                    nc.tensor.matmul(rsps[:, ds(sc, 1)], gaT[:, sc],
                                     gse_sb[:], start=False,
                                     stop=(sc == SCH - 1))
                nc.vector.tensor_copy(rs_sb[:], rsps)
                nc.vector.reciprocal(rcp_sb[:], rs_sb[:])
            nc.vector.tensor_scalar_mul(t1_sb[:], w1b_sb[:, 0:EMBED],
                                        gsa_sb[:])
            nc.vector.affine_then_add(m_sb[0:NSPL, ts(0, 512)], mps[0],
                                      t1_sb[:, ts(0, 512)], ampc_sb, 0.0)
            with tc.tile_pool(name="ups", bufs=6, space="PSUM") as ups, \
                 tc.tile_pool(name="yb", bufs=6) as yb:
                for ec in range(KC):
                    nc.tensor.matmul(mps[1], ht_sb[:, ec],
                                     wo_sb[:, ec, ts(1, 512)],
                                     start=(ec == 0), stop=(ec == KC - 1))
                # ua (low half of y) + its scale overlap the mps[1] accum;
                # scales spread over ACT/DVE/Pool so no one engine paces the
                # tail; y DMAs alternate the SP and Pool queues
                ysbs = []
                for sc in range(SCH):
                    ua = ups.tile([P, 512], f32, tag="ups", name=f"ua{sc}")
                    nc.tensor.matmul(ua, gaT[:, sc], m_sb[:, ts(0, 512)],
                                     start=True, stop=True)
                    ysb = yb.tile([P, EMBED], bf16, tag="ysb")
                    if sc % 2 == 0:
                        nc.scalar.activation(ysb[:, ts(0, 512)], ua, AF.Copy,
                                             scale=rcp_sb[:, ds(sc, 1)])
                    else:
                        nc.vector.tensor_scalar_mul(ysb[:, ts(0, 512)], ua,
                                                    rcp_sb[:, ds(sc, 1)])
                    ysbs.append(ysb)
                nc.vector.affine_then_add(m_sb[0:NSPL, ts(1, 512)], mps[1],
                                          t1_sb[:, ts(1, 512)], ampc_sb, 0.0)
                for sc in range(SCH):
                    ub = ups.tile([P, 512], f32, tag="ups", name=f"ub{sc}")
                    nc.tensor.matmul(ub, gaT[:, sc], m_sb[:, ts(1, 512)],
                                     start=True, stop=True)
                    ysb = ysbs[sc]
                    if sc % 2 == 0:
                        nc.vector.tensor_scalar_mul(ysb[:, ts(1, 512)], ub,
                                                    rcp_sb[:, ds(sc, 1)])
                    else:
                        nc.scalar.activation(ysb[:, ts(1, 512)], ub, AF.Copy,
                                             scale=rcp_sb[:, ds(sc, 1)])
                    ydma = nc.sync if sc % 2 == 0 else nc.gpsimd
                    ydma.dma_start(yr[sc], ysb)
        cpool_cm.__exit__(None, None, None)

    nc.finalize()
    return nc


def _prep_inputs(query, key, value, Wq, bq, Wk, bk, Wv, bv, Wo, bo,
                 splat_centers, splat_log_scales, splat_amplitudes):
    """Build the 8 per-core input maps (host-side sharding/layout prep)."""
    f = np.float32
    q = np.asarray(query, f)
    v = np.asarray(value, f)
    Wq = np.asarray(Wq, f); bq = np.asarray(bq, f)
    Wv = np.asarray(Wv, f); bv = np.asarray(bv, f)
    Wo = np.asarray(Wo, f); bo = np.asarray(bo, f)
    C = np.asarray(splat_centers, f)
    ls = np.asarray(splat_log_scales, f)
    amp = np.asarray(splat_amplitudes, f)

    wqT = np.ascontiguousarray(Wq.T).astype(BF16)
    wvT = np.ascontiguousarray(Wv.T).astype(BF16)
    woT = np.ascontiguousarray(Wo.T).astype(BF16)
    bq2 = np.ascontiguousarray(bq.reshape(KC, P).T)
    inv2v = (0.5 * np.exp(-2.0 * ls)).astype(np.float64)
    c2 = (C.astype(np.float64) ** 2).sum(1)
    # exponent arg = -inv2v*d2 = (2*inv2v)*q.c + (-inv2v)*|q|^2 + (-inv2v*c2)
    cts = np.ascontiguousarray((2.0 * inv2v[:, None] * C).T).astype(BF16)
    w1 = (Wo.astype(np.float64) @ bv.astype(np.float64) + bo).astype(f)
    w1b = np.empty((NSPL, EMBED + 1), f)
    w1b[:, 0:EMBED] = w1[None, :]
    w1b[:, EMBED] = amp.astype(f)
    # eps*bo as bf16 hi + lo (residual) rows: ~16 mantissa bits combined
    ebo = (EPS * bo).astype(f)
    ehi = ebo.astype(BF16)
    elo = (ebo - ehi.astype(f)).astype(BF16)
    epsbo = np.ascontiguousarray(np.stack([ehi, elo]))

    # blob_b [128, 193] bf16: o64s(64) | id128(128) | onecol(1)
    blob_b = np.empty((P, 193), BF16)
    blob_b[:, 0:NSPL] = (-inv2v).astype(f)[None, :]
    blob_b[:, NSPL:NSPL + P] = np.eye(P, dtype=BF16)
    blob_b[:, 192] = 1.0
    # blob1b [1, 648] bf16: ones(128) | epsrow(8) | zeros(512)
    blob1b = np.zeros((1, 648), BF16)
    blob1b[0, 0:P] = 1.0
    blob1b[0, P:P + SCH] = EPS
    # blob1f [1, 641] f32: cb1w(512) | ones(128) | one(1)
    blob1f = np.empty((1, 641), f)
    blob1f[0, 0:512] = np.tile((-inv2v * c2).astype(f), SCH)
    blob1f[0, 512:641] = 1.0

    shared = dict(wqT=wqT, wvT=wvT, woT=woT, bq2=bq2, cts=cts,
                  blob_b=blob_b, blob1b=blob1b, blob1f=blob1f,
                  w1b=w1b, epsbo=epsbo)
    in_maps = []
    for c in range(NCORES):
        b, h = c // 2, c % 2
        # roll the sequence axis so own rows are always 0..1023
        qb = np.concatenate([q[b, h * SOWN:], q[b, :h * SOWN]], axis=0)
        vb = np.concatenate([v[b, h * SOWN:], v[b, :h * SOWN]], axis=0)
        m = dict(shared)
        m["xqT"] = np.ascontiguousarray(qb.T).astype(BF16)
        m["vrl"] = np.ascontiguousarray(vb).astype(BF16)
        in_maps.append(m)
    return in_maps


def run_cores(inputs, trace=False):
    """Run the SPMD kernel; returns (full_output, BassKernelResults)."""
    global _PROG
    from concourse.bass_utils import run_bass_kernel_spmd
    if _PROG is None:
        _PROG = _build_program()
    nc = _PROG
    in_maps = _prep_inputs(**inputs)
    res = run_bass_kernel_spmd(nc, in_maps, list(range(NCORES)), trace=trace)
    out = np.empty((B, S, EMBED), np.float32)
    for c in range(NCORES):
        b, h = c // 2, c % 2
        out[b, h * SOWN:(h + 1) * SOWN] = res.results[c]["y"].astype(np.float32)
    return out, res


def kernel(**inputs):
    out, _ = run_cores(inputs, trace=False)
    return out


# revision 55
# speedup vs baseline: 3.4865x; 1.0176x over previous
"""HSA (hierarchical splat attention) Bass kernel for Trainium2, 8 NeuronCores.

Math (per batch b):
    q = query @ Wq.T + bq                      [S, D]
    d2[s,n]  = |q_s|^2 - 2 q_s.c_n + |c_n|^2
    G[s,n]   = exp(-d2[s,n] * inv2v[n]),  inv2v = 0.5*exp(-2*log_scales)
    A        = (G diag(amp) G^T) row-normalized (+eps)
    out      = A @ (value @ Wv.T + bv) ;  y = out @ Wo.T + bo

Because A = G diag(a) G^T is rank-64, A is never materialized:
    gsum[n]  = sum_t G[t,n]
    Hraw     = G^T @ value                       [N, D]
    M        = Hraw @ Wv.T @ Wo.T + gsum (x) w1  [N, D],  w1 = Wo@bv + bo
    rs[s]    = (amp*G)[s,:] @ gsum  (+ eps)
    y[s,:]   = ((amp*G)[s,:] @ M + eps*bo) / rs[s]
The eps*bo term makes the G-underflow case exact: rs=eps, y=bo.
bo and eps ride inside the matmuls via an appended ones-row in Ga
(row 64) matching an eps*bo row in M.

Sharding: 8 cores = (batch b = c//2, seq-half h = c%2), no collectives.
Each core computes full-batch q-proj/G (needed for gsum/Hraw) and its own
1024 output rows. The sequence axis is rolled per-core so own rows are
always t-chunks 0..7 (valid: the t-contractions are permutation-invariant).

Device dataflow (matmul = lhsT.T @ rhs, contraction on partitions):
  qT[e,s]    : lhsT=wq chunk, rhs=xq chunk          (accum over d)
  d2t[t,n]   : psum [128,8,64] x2; bank-wide K=1 init matmul injects
               -inv2v*c2 (start=True), then per e: lhsT=qe[:,tb] rhs=cts
               (+2*inv2v*q.c) and lhsT=sq[:,tb] rhs=o64s (-inv2v*|q|^2)
  G,Ga       : ACT exp (one [128,512] op per tile); Ga = G * ampw (DVE)
  gsum       : lhsT=ones col, rhs=G t-chunks -> [1,64]; PE-transpose -> [64,1]
  HrawT[e,n] : lhsT=vrl[t, e-chunk], rhs=G[t] t-chunk   (accum over t)
  HT[e',n]   : lhsT=wvT chunk,  rhs=HrawT chunk         (accum over d)
  M[n,e']    : lhsT=HT chunk,   rhs=woT chunk           (accum over e')
               + rank-1 gsum (x) w1 via DVE; -> m_sb [65,1024] (row64=eps*bo)
  rs         : psum [128,8]; K=1 init=eps; lhsT=GaT[:,sc], rhs=gsum col
  U,y        : lhsT=GaT[:,sc] [65,128], rhs=m_sb -> U; y = U * recip(rs)
"""

import numpy as np
import ml_dtypes

BF16 = ml_dtypes.bfloat16
EMBED = 1024
S = 2048
NSPL = 64
B = 4
NCORES = 8
P = 128
KC = EMBED // P   # 8 contraction chunks over d/e
TCH = S // P      # 16 t-chunks
SOWN = S // 2     # 1024 own output rows per core
SCH = SOWN // P   # 8
EPS = 1e-8

_PROG = None  # cached program


def _build_program():
    import concourse.bass as bass
    import concourse.mybir as mybir
    from concourse import bacc
    from concourse.tile import TileContext
    from concourse.bass import ts, ds

    f32 = mybir.dt.float32
    bf16 = mybir.dt.bfloat16
    AF = mybir.ActivationFunctionType

    nc = bacc.Bacc("TRN2", target_bir_lowering=False, debug=False)
    xqT = nc.declare_dram_parameter("xqT", [EMBED, S], bf16, isOutput=False)
    vrl = nc.declare_dram_parameter("vrl", [S, EMBED], bf16, isOutput=False)
    wqT = nc.declare_dram_parameter("wqT", [EMBED, EMBED], bf16, isOutput=False)
    wvT = nc.declare_dram_parameter("wvT", [EMBED, EMBED], bf16, isOutput=False)
    woT = nc.declare_dram_parameter("woT", [EMBED, EMBED], bf16, isOutput=False)
    cts = nc.declare_dram_parameter("cts", [EMBED, NSPL], bf16, isOutput=False)
    bq2 = nc.declare_dram_parameter("bq2", [P, KC], f32, isOutput=False)
    # packed constants: fewer DMA instructions (HWDGE serializes per-DMA)
    # blob_b [128, 193] bf16: o64s(64) | id128(128) | onecol(1)
    blob_b = nc.declare_dram_parameter("blob_b", [P, 193], bf16, isOutput=False)
    # blob1b [1, 1160] bf16: ones(128) | epsrow(8) | zeros(512) |
    #                        cb1w_hi(256) | cb1w_lo(256)
    blob1b = nc.declare_dram_parameter("blob1b", [1, 1160], bf16,
                                       isOutput=False)
    # blob1f [1, 641] f32: cb1w(512) | ones(128) | one(1)
    blob1f = nc.declare_dram_parameter("blob1f", [1, 641], f32, isOutput=False)
    # w1b [64, 1025] f32: broadcast (Wo@bv + bo) | amp column
    w1b = nc.declare_dram_parameter("w1b", [NSPL, EMBED + 1], f32,
                                    isOutput=False)
    # eps*bo split hi/lo so the bf16 rank-1 rows carry ~16 mantissa bits
    epsbo = nc.declare_dram_parameter("epsbo", [2, EMBED], bf16, isOutput=False)
    y = nc.declare_dram_parameter("y", [SOWN, EMBED], bf16, isOutput=True)

    with TileContext(nc) as tc:
        cpool_cm = tc.tile_pool(name="const", bufs=1)
        cpool = cpool_cm.__enter__()
        bq_sb = cpool.tile([P, KC], f32)
        cts_sb = cpool.tile([P, KC, NSPL], bf16)
        bb_sb = cpool.tile([P, 193], bf16)
        b1b_sb = cpool.tile([1, 1160], bf16)
        b1f_sb = cpool.tile([1, 641], f32)
        w1b_sb = cpool.tile([NSPL, EMBED + 1], f32)
        sqacc = cpool.tile([P, S], bf16)           # sum of qe^2 over e-chunks
        gts = cpool.tile([P, TCH, NSPL], bf16)     # G in [t, n] layout
        gaT = cpool.tile([NSPL + 2, SCH, P], bf16)  # G^T own rows + ones rows
        vrl_sb = cpool.tile([P, TCH, EMBED], bf16)
        wv_sb = cpool.tile([P, KC, EMBED], bf16)
        wo_sb = cpool.tile([P, KC, EMBED], bf16)
        hrawT_sb = cpool.tile([P, KC, NSPL], bf16)
        ht_sb = cpool.tile([P, KC, NSPL], bf16)
        m_sb = cpool.tile([NSPL + 2, EMBED], bf16)
        t1_sb = cpool.tile([NSPL, EMBED], f32)
        gsum_sb = cpool.tile([1, NSPL], f32)
        gsumc_sb = cpool.tile([NSPL, 1], f32)
        gsa_sb = cpool.tile([NSPL, 1], f32)        # amp * gsum
        gse_sb = cpool.tile([NSPL + 2, 1], bf16)
        rs_sb = cpool.tile([P, SCH], f32)
        rcp_sb = cpool.tile([P, SCH], f32)

        # const views into packed blobs
        o64s_sb = bb_sb[:, 0:NSPL]
        id_sb = bb_sb[:, NSPL:NSPL + P]
        oncl_sb = bb_sb[:, 192:193]
        on1b_sb = b1b_sb[:, 0:P]
        epsr_sb = b1b_sb[:, P:P + SCH]
        zrow_sb = b1b_sb[:, 136:648]
        cbhi_sb = b1b_sb[:, 648:904]
        cblo_sb = b1b_sb[:, 904:1160]
        o11f_sb = b1f_sb[:, 640:641]
        ampc_sb = w1b_sb[:, EMBED:EMBED + 1]

        # ---------------- Phase A: q projection + d2 in [t, n] ----------------
        with tc.tile_pool(name="pa", bufs=1) as pa, \
             tc.tile_pool(name="qe", bufs=2) as qep, \
             tc.tile_pool(name="sqe", bufs=2) as sqp, \
             tc.tile_pool(name="psq", bufs=4, space="PSUM") as psq, \
             tc.tile_pool(name="psd", bufs=1, space="PSUM") as psd:
            xq = pa.tile([P, KC, S], bf16)
            wq = pa.tile([P, KC, EMBED], bf16)
            wqr = wqT.rearrange("(k p) e -> k p e", p=P)
            xqr = xqT.rearrange("(k p) s -> k p s", p=P)
            # critical-path chunks spread over the SP/Activation/Pool queues
            # (v1 charges transfer time to the issuing engine); k=0 split
            # into small pieces so the first matmuls start ASAP
            nc.sync.dma_start(wq[:, 0, 0:512], wqr[0][:, 0:512])
            nc.gpsimd.dma_start(xq[:, 0, 0:512], xqr[0][:, 0:512])
            nc.sync.dma_start(wq[:, 0, 512:EMBED], wqr[0][:, 512:EMBED])
            nc.sync.dma_start(xq[:, 0, 512:S], xqr[0][:, 512:S])
            # b1b early: the d2 psum-init matmuls read cb1w hi/lo from it
            nc.gpsimd.dma_start(b1b_sb[:], blob1b[:])
            qeng = {1: nc.scalar, 2: nc.gpsimd, 3: nc.sync, 4: nc.scalar,
                    5: nc.gpsimd, 6: nc.sync, 7: nc.scalar}
            for k in range(1, KC):
                qeng[k].dma_start(wq[:, k], wqr[k])
                qeng[k].dma_start(xq[:, k], xqr[k])
            # remaining constants + bulk prefetch on the Pool engine
            nc.gpsimd.dma_start(bq_sb[:], bq2[:])
            nc.gpsimd.dma_start(cts_sb[:], cts.rearrange("(k p) n -> p k n", p=P))
            nc.gpsimd.dma_start(bb_sb[:], blob_b[:])
            nc.gpsimd.dma_start(b1f_sb[:], blob1f[:])
            nc.gpsimd.dma_start(m_sb[NSPL:NSPL + 2, :], epsbo[:])
            nc.gpsimd.dma_start(w1b_sb[:], w1b[:])
            nc.gpsimd.dma_start(vrl_sb[:], vrl.rearrange("(t p) e -> p t e", p=P))
            nc.gpsimd.dma_start(wv_sb[:], wvT.rearrange("(k p) e -> p k e", p=P))
            nc.gpsimd.dma_start(wo_sb[:], woT.rearrange("(k p) e -> p k e", p=P))

            d2a = [psd.tile([P, 4, NSPL], f32, name=f"d2a{i}")
                   for i in range(4)]
            # bank-wide group init: fills each d2 bank with -inv2v*c2 via
            # K=1 bf16 hi+lo matmuls (bf16 pair carries ~16 mantissa bits)
            for i in range(4):
                nc.tensor.matmul(d2a[i][:, :, :], on1b_sb[:], cbhi_sb,
                                 start=True, stop=False)
                nc.tensor.matmul(d2a[i][:, :, :], on1b_sb[:], cblo_sb,
                                 start=False, stop=False)

            prev = None  # software pipelining: d2(e-1) emitted after qps(e)
            for e in range(KC):
                qps = [psq.tile([P, 512], f32, tag="qps", name=f"qps{e}_{i}")
                       for i in range(4)]
                for k in range(KC):
                    for s4 in range(4):
                        nc.tensor.matmul(
                            qps[s4], wq[:, k, ts(e, P)], xq[:, k, ts(s4, 512)],
                            start=(k == 0), stop=(k == KC - 1))
                if prev is not None:
                    qep_prev, eprev = prev
                    # ct-term only; |q|^2 accumulates on DVE into sqacc
                    for tb in range(TCH):
                        sl = d2a[tb // 4][:, tb % 4]
                        nc.tensor.matmul(sl, qep_prev[:, ts(tb, P)],
                                         cts_sb[:, eprev], start=False,
                                         stop=False)
                qe = qep.tile([P, S], bf16, tag="qe")
                for s4 in range(4):
                    if s4 % 2 == 0:
                        nc.scalar.activation(qe[:, ts(s4, 512)], qps[s4],
                                             AF.Identity, bias=bq_sb[:, ds(e, 1)])
                    else:
                        nc.vector.tensor_scalar_add(qe[:, ts(s4, 512)], qps[s4],
                                                    bq_sb[:, ds(e, 1)])
                if e < KC - 1:
                    if e == 0:
                        nc.vector.tensor_mul(sqacc[:], qe, qe)
                    else:
                        sq = sqp.tile([P, S], bf16, tag="sq")
                        nc.vector.tensor_mul(sq, qe, qe)
                        nc.vector.tensor_add(sqacc[:], sqacc[:], sq)
                else:
                    sq7 = sqp.tile([P, S], bf16, tag="sq")
                    nc.vector.tensor_mul(sq7[:, 0:1024], qe[:, 0:1024],
                                         qe[:, 0:1024])
                    nc.vector.tensor_mul(sq7[:, 1024:S], qe[:, 1024:S],
                                         qe[:, 1024:S])
                prev = (qe, e)
            qe, e = prev
            # ct + sqacc terms first (independent of sq7), then the sq7
            # ones-term; each quarter-tile stops early so its exp overlaps
            for tb in range(TCH):
                sl = d2a[tb // 4][:, tb % 4]
                nc.tensor.matmul(sl, qe[:, ts(tb, P)], cts_sb[:, e],
                                 start=False, stop=False)
                nc.tensor.matmul(sl, sqacc[:, ts(tb, P)], o64s_sb[:],
                                 start=False, stop=False)
            for tb in range(TCH):
                sl = d2a[tb // 4][:, tb % 4]
                nc.tensor.matmul(sl, sq7[:, ts(tb, P)], o64s_sb[:],
                                 start=False, stop=(tb % 4 == 3))
                if tb % 4 == 3:
                    nc.scalar.activation(gts[:, tb - 3:tb + 1],
                                         d2a[tb // 4][:], AF.Exp)

        # ---------------- Phase B: H chain, gsum, G^T ----------------
        # amp is folded into M's rows and into gsum (U = G @ diag(amp) @ M),
        # so G^T transposes run straight off the exps with no amp multiply.
        # PE emission order keeps the engine hot: HrawT g0 -> gsum/transposes
        # -> HrawT g1; rs waits on a DVE chain so it moves to Phase C.
        with tc.tile_pool(name="gat", bufs=2, space="PSUM") as gat, \
             tc.tile_pool(name="gsp", bufs=1, space="PSUM") as gsp, \
             tc.tile_pool(name="hrt", bufs=2, space="PSUM") as hrt:
            nc.gpsimd.memset(gaT[NSPL:NSPL + 2], 1.0)
            gsps = gsp.tile([1, NSPL], f32, name="gsps")
            gscps = gsp.tile([NSPL, 1], f32, name="gscps")
            for g in range(2):
                # HrawT[e,n] = sum_t value[t,e] G[t,n]; 4 e-chunks per bank
                h = hrt.tile([P, 4, NSPL], f32, tag="hrt")
                nc.tensor.matmul(h[:, :, :], on1b_sb[:], zrow_sb[:, 0:256],
                                 start=True, stop=False)
                for i in range(4):
                    e = g * 4 + i
                    for t in range(TCH):
                        nc.tensor.matmul(h[:, i], vrl_sb[:, t, ts(e, P)],
                                         gts[:, t], start=False,
                                         stop=(i == 3 and t == TCH - 1))
                    if g == 0 and i == 0:
                        # interleave work with matching exp-quarter deps so
                        # PE isn't gated by the serial exp stream
                        for q in range(4):
                            for tb in range(q * 4, q * 4 + 4):
                                nc.tensor.matmul(gsps, oncl_sb[:],
                                                 gts[:, tb], start=(tb == 0),
                                                 stop=(tb == TCH - 1))
                            if q < 2:
                                for sc in range(q * 4, q * 4 + 4):
                                    gatp = gat.tile([NSPL, P], bf16,
                                                    tag="gat")
                                    nc.tensor.transpose(gatp, gts[:, sc],
                                                        id_sb[:])
                                    if sc % 2 == 0:
                                        nc.vector.tensor_copy(
                                            gaT[0:NSPL, sc], gatp)
                                    else:
                                        nc.scalar.activation(
                                            gaT[0:NSPL, sc], gatp, AF.Copy)
                        nc.scalar.activation(gsum_sb[:], gsps, AF.Copy)
                        nc.tensor.matmul(gscps, gsum_sb[:], o11f_sb[:],
                                         start=True, stop=True)
                if g == 0:
                    nc.scalar.activation(hrawT_sb[:, 0:4], h, AF.Copy)
                else:
                    nc.vector.tensor_copy(hrawT_sb[:, 4:KC], h)
            nc.vector.tensor_copy(gsumc_sb[:], gscps)
            nc.vector.tensor_mul(gsa_sb[:], gsumc_sb[:], ampc_sb)
            nc.gpsimd.memset(gse_sb[:], 0.0)
            nc.vector.tensor_copy(gse_sb[0:NSPL], gsa_sb[:])

        # ---------------- Phase C: HT, M, rs, U, y ----------------
        yr = y.rearrange("(c p) e -> c p e", p=P)
        with tc.tile_pool(name="mp", bufs=1, space="PSUM") as mp:
            mps = [mp.tile([NSPL, 512], f32, name=f"mps{j}") for j in range(2)]
            with tc.tile_pool(name="htp", bufs=2, space="PSUM") as htp, \
                 tc.tile_pool(name="rsp", bufs=1, space="PSUM") as rsp:
                # HT[e',n] = sum_d Wv[e',d] HrawT[d,n]; 4 e'-chunks per bank
                for g in range(2):
                    h2 = htp.tile([P, 4, NSPL], f32, tag="htp")
                    nc.tensor.matmul(h2[:, :, :], on1b_sb[:],
                                     zrow_sb[:, 0:256], start=True, stop=False)
                    for i in range(4):
                        ec = g * 4 + i
                        for d in range(KC):
                            nc.tensor.matmul(h2[:, i], wv_sb[:, d, ts(ec, P)],
                                             hrawT_sb[:, d], start=False,
                                             stop=(i == 3 and d == KC - 1))
                    if g == 0:
                        nc.scalar.activation(ht_sb[:, 0:4], h2, AF.Copy)
                    else:
                        nc.vector.tensor_copy(ht_sb[:, 4:KC], h2)
                    # M low half accumulates as HT chunks land
                    for i in range(4):
                        ec = g * 4 + i
                        nc.tensor.matmul(mps[0], ht_sb[:, ec],
                                         wo_sb[:, ec, ts(0, 512)],
                                         start=(ec == 0), stop=(ec == KC - 1))
                # rs = G @ (amp*gsum) (+eps via init matmul)
                rsps = rsp.tile([P, SCH], f32, name="rsps")
                nc.tensor.matmul(rsps, on1b_sb[:], epsr_sb[:], start=True,
                                 stop=False)
                for sc in range(SCH):
                    nc.tensor.matmul(rsps[:, ds(sc, 1)], gaT[:, sc],
                                     gse_sb[:], start=False,
                                     stop=(sc == SCH - 1))
                nc.vector.tensor_copy(rs_sb[:], rsps)
                nc.vector.reciprocal(rcp_sb[:], rs_sb[:])
            nc.vector.tensor_scalar_mul(t1_sb[:], w1b_sb[:, 0:EMBED],
                                        gsa_sb[:])
            nc.vector.affine_then_add(m_sb[0:NSPL, ts(0, 512)], mps[0],
                                      t1_sb[:, ts(0, 512)], ampc_sb, 0.0)
            with tc.tile_pool(name="ups", bufs=6, space="PSUM") as ups, \
                 tc.tile_pool(name="yb", bufs=6) as yb:
                for ec in range(KC):
                    nc.tensor.matmul(mps[1], ht_sb[:, ec],
                                     wo_sb[:, ec, ts(1, 512)],
                                     start=(ec == 0), stop=(ec == KC - 1))
                # ua (low half of y) + its scale overlap the mps[1] accum;
                # scales spread over ACT/DVE/Pool so no one engine paces the
                # tail; y DMAs alternate the SP and Pool queues
                ysbs = []
                for sc in range(SCH):
                    ua = ups.tile([P, 512], f32, tag="ups", name=f"ua{sc}")
                    nc.tensor.matmul(ua, gaT[:, sc], m_sb[:, ts(0, 512)],
                                     start=True, stop=True)
                    ysb = yb.tile([P, EMBED], bf16, tag="ysb")
                    if sc % 2 == 0:
                        nc.scalar.activation(ysb[:, ts(0, 512)], ua, AF.Copy,
                                             scale=rcp_sb[:, ds(sc, 1)])
                    else:
                        nc.vector.tensor_scalar_mul(ysb[:, ts(0, 512)], ua,
                                                    rcp_sb[:, ds(sc, 1)])
                    ydma = nc.sync if sc % 2 == 0 else nc.gpsimd
                    ydma.dma_start(yr[sc][:, 0:512], ysb[:, ts(0, 512)])
                    ysbs.append(ysb)
                nc.vector.affine_then_add(m_sb[0:NSPL, ts(1, 512)], mps[1],
                                          t1_sb[:, ts(1, 512)], ampc_sb, 0.0)
                for sc in range(SCH):
                    ub = ups.tile([P, 512], f32, tag="ups", name=f"ub{sc}")
                    nc.tensor.matmul(ub, gaT[:, sc], m_sb[:, ts(1, 512)],
                                     start=True, stop=True)
                    ysb = ysbs[sc]
                    if sc % 2 == 0:
                        nc.vector.tensor_scalar_mul(ysb[:, ts(1, 512)], ub,
                                                    rcp_sb[:, ds(sc, 1)])
                    else:
                        nc.scalar.activation(ysb[:, ts(1, 512)], ub, AF.Copy,
                                             scale=rcp_sb[:, ds(sc, 1)])
                    ydma = nc.sync if sc % 2 == 1 else nc.gpsimd
                    ydma.dma_start(yr[sc][:, 512:EMBED], ysb[:, ts(1, 512)])
        cpool_cm.__exit__(None, None, None)

    nc.finalize()
    return nc


def _prep_inputs(query, key, value, Wq, bq, Wk, bk, Wv, bv, Wo, bo,
                 splat_centers, splat_log_scales, splat_amplitudes):
    """Build the 8 per-core input maps (host-side sharding/layout prep)."""
    f = np.float32
    q = np.asarray(query, f)
    v = np.asarray(value, f)
    Wq = np.asarray(Wq, f); bq = np.asarray(bq, f)
    Wv = np.asarray(Wv, f); bv = np.asarray(bv, f)
    Wo = np.asarray(Wo, f); bo = np.asarray(bo, f)
    C = np.asarray(splat_centers, f)
    ls = np.asarray(splat_log_scales, f)
    amp = np.asarray(splat_amplitudes, f)

    wqT = np.ascontiguousarray(Wq.T).astype(BF16)
    wvT = np.ascontiguousarray(Wv.T).astype(BF16)
    woT = np.ascontiguousarray(Wo.T).astype(BF16)
    bq2 = np.ascontiguousarray(bq.reshape(KC, P).T)
    inv2v = (0.5 * np.exp(-2.0 * ls)).astype(np.float64)
    c2 = (C.astype(np.float64) ** 2).sum(1)
    # exponent arg = -inv2v*d2 = (2*inv2v)*q.c + (-inv2v)*|q|^2 + (-inv2v*c2)
    cts = np.ascontiguousarray((2.0 * inv2v[:, None] * C).T).astype(BF16)
    w1 = (Wo.astype(np.float64) @ bv.astype(np.float64) + bo).astype(f)
    w1b = np.empty((NSPL, EMBED + 1), f)
    w1b[:, 0:EMBED] = w1[None, :]
    w1b[:, EMBED] = amp.astype(f)
    # eps*bo as bf16 hi + lo (residual) rows: ~16 mantissa bits combined
    ebo = (EPS * bo).astype(f)
    ehi = ebo.astype(BF16)
    elo = (ebo - ehi.astype(f)).astype(BF16)
    epsbo = np.ascontiguousarray(np.stack([ehi, elo]))

    # blob_b [128, 193] bf16: o64s(64) | id128(128) | onecol(1)
    blob_b = np.empty((P, 193), BF16)
    blob_b[:, 0:NSPL] = (-inv2v).astype(f)[None, :]
    blob_b[:, NSPL:NSPL + P] = np.eye(P, dtype=BF16)
    blob_b[:, 192] = 1.0
    # blob1b [1, 1160] bf16: ones(128) | epsrow(8) | zeros(512) |
    #                        cb1w_hi(256) | cb1w_lo(256)
    cb = np.tile((-inv2v * c2).astype(f), 4)       # [256] = 4 quarter-slices
    cbhi = cb.astype(BF16)
    cblo = (cb - cbhi.astype(f)).astype(BF16)
    blob1b = np.zeros((1, 1160), BF16)
    blob1b[0, 0:P] = 1.0
    blob1b[0, P:P + SCH] = EPS
    blob1b[0, 648:904] = cbhi
    blob1b[0, 904:1160] = cblo
    # blob1f [1, 641] f32: cb1w(512) | ones(128) | one(1)
    blob1f = np.empty((1, 641), f)
    blob1f[0, 0:512] = np.tile((-inv2v * c2).astype(f), SCH)
    blob1f[0, 512:641] = 1.0

    shared = dict(wqT=wqT, wvT=wvT, woT=woT, bq2=bq2, cts=cts,
                  blob_b=blob_b, blob1b=blob1b, blob1f=blob1f,
                  w1b=w1b, epsbo=epsbo)
    in_maps = []
    for c in range(NCORES):
        b, h = c // 2, c % 2
        # roll the sequence axis so own rows are always 0..1023
        qb = np.concatenate([q[b, h * SOWN:], q[b, :h * SOWN]], axis=0)
        vb = np.concatenate([v[b, h * SOWN:], v[b, :h * SOWN]], axis=0)
        m = dict(shared)
        m["xqT"] = np.ascontiguousarray(qb.T).astype(BF16)
        m["vrl"] = np.ascontiguousarray(vb).astype(BF16)
        in_maps.append(m)
    return in_maps


def run_cores(inputs, trace=False):
    """Run the SPMD kernel; returns (full_output, BassKernelResults)."""
    global _PROG
    from concourse.bass_utils import run_bass_kernel_spmd
    if _PROG is None:
        _PROG = _build_program()
    nc = _PROG
    in_maps = _prep_inputs(**inputs)
    res = run_bass_kernel_spmd(nc, in_maps, list(range(NCORES)), trace=trace)
    out = np.empty((B, S, EMBED), np.float32)
    for c in range(NCORES):
        b, h = c // 2, c % 2
        out[b, h * SOWN:(h + 1) * SOWN] = res.results[c]["y"].astype(np.float32)
    return out, res


def kernel(**inputs):
    out, _ = run_cores(inputs, trace=False)
    return out
